# revision 1
# baseline (speedup 1.0000x reference)
"""Trainium2 Bass kernel for a MinkowskiNet BasicBlock:
    out = relu(bn2(conv(relu(bn1(conv(x, w1))), w2)) + x)
with gather-GEMM-scatter sparse convolutions over (in_map, out_map) pair lists.

Strategy (8 NeuronCores, SPMD):
  - Shard by output-voxel owner: core c owns output rows [c*S, (c+1)*S), S = N/8.
  - Replicate x (as a bf16 hi/lo split table, 256B/row) and weights to all cores.
  - Gather rows with dma_gather(transpose=True): channels land on partitions,
    which is exactly the matmul lhsT layout.  hi/lo split gives ~fp32 accuracy
    from bf16 matmuls (2 PSUM-accumulated matmuls per token tile).
  - Scatter-add with dma_scatter_add (CCE f32).  The CCE add is NOT atomic
    across SDMA engines, so duplicate target rows inside one call race.
    Fix: windows of 2048 tokens rotate over B=4 accumulator buffers (calls on
    the same buffer are WAW-serialized by Tile; different buffers never share
    addresses), and within a window duplicate rows are redirected to unique
    aux rows in the buffer tail; aux rows are folded back by recursive
    cleanup scatter passes (host-precomputed, fully static).
  - BN folded: scale into weights (host side), bias added on-chip post-scatter.
  - Intermediate activation re-split to bf16 hi/lo and AllGather'd across cores.
  - int16 gather indices -> gather per (k, input-chunk-of-S-rows) groups.
"""

import sys

if "/opt/trn_rl_repo" not in sys.path:
    sys.path.insert(0, "/opt/trn_rl_repo")

import numpy as np
import ml_dtypes

BF16 = ml_dtypes.bfloat16

# ---------------------------------------------------------------- problem cfg
N = 200000  # voxels
C = 64      # channels
K = 27      # kernel offsets
M = 100000  # pairs per offset
W = 8       # cores
EPS = 1e-5

WTOK = 2048   # tokens per scatter window (one dma_scatter_add call)
TILE = 128    # tokens per matmul tile
NBUF = 4      # rotating scatter accumulator buffers


# ---------------------------------------------------------------- host-side prep
def _split_hi_lo(a):
    hi = a.astype(BF16)
    lo = (a.astype(np.float32) - hi.astype(np.float32)).astype(BF16)
    return hi, lo


def _make_split_table(a_f32):
    """[R, C] f32 -> [R, 2C] bf16, row = [hi(C) | lo(C)] (256B rows for C=64)."""
    hi, lo = _split_hi_lo(a_f32)
    return np.concatenate([hi, lo], axis=1)


def _weight_stacks(w_scaled):
    """[K, C, C] f32 -> (Ra, Rb) [K, 2C, C] bf16 with Ra=[wh;wh], Rb=[wl;wl]."""
    wh, wl = _split_hi_lo(w_scaled)
    return (np.ascontiguousarray(np.concatenate([wh, wh], axis=1)),
            np.ascontiguousarray(np.concatenate([wl, wl], axis=1)))


def _pad128(n):
    return ((int(n) + 127) // 128) * 128


def _prep_indices_static(in_map, out_map, n, w):
    """Deterministic two-pass version: aux rows laid out per (level, buffer)
    with uniform capacities so the device program is core-independent."""
    S = n // w
    kk = in_map.shape[0]
    TRASH = S

    owner = out_map // S
    chunk = in_map // S
    counts = np.zeros((w, kk, w), dtype=np.int64)
    for k in range(kk):
        flat = owner[k] * w + chunk[k]
        counts[:, k, :] = np.bincount(flat, minlength=w * w).reshape(w, w)
    slot_sizes = np.maximum(((counts.max(axis=0) + 127) // 128) * 128, 128)
    tot = int(slot_sizes.sum())
    n_win = (tot + WTOK - 1) // WTOK

    g_all, s_raw = [], []
    for c in range(w):
        g_stream = np.zeros(tot, dtype=np.int32)
        s_stream = np.full(tot, TRASH, dtype=np.int32)
        off = 0
        for k in range(kk):
            sel_c = owner[k] == c
            i_k = in_map[k][sel_c]
            o_k = out_map[k][sel_c] - c * S
            ch_k = chunk[k][sel_c]
            for j in range(w):
                L = int(slot_sizes[k, j])
                selj = ch_k == j
                i_loc = i_k[selj] - j * S
                o_loc = o_k[selj]
                order = np.argsort(o_loc, kind="stable")
                cnt = len(i_loc)
                g_stream[off:off + cnt] = i_loc[order]
                s_stream[off:off + cnt] = o_loc[order]
                off += L
        g_all.append(g_stream)
        s_raw.append(s_stream)

    # ---- iterative dedup with per-level uniform capacities ----
    # level 0 = main stream; dups of level l become level l+1 tokens.
    # per core state
    streams = [[s] for s in s_raw]             # per core: [lvl0, lvl1, ...]
    pend = [None] * w                          # per core: list[(buf, true_r)]
    lev_caps = []                              # per level: [cap_b] * NBUF
    widx0 = 0
    lvl = 0
    cur_len = tot
    while True:
        # dedup current level (stream index lvl) for each core
        n_w = (cur_len + WTOK - 1) // WTOK
        for c in range(w):
            st = streams[c][lvl]
            pc = []
            for wi in range(n_w):
                buf = (widx0 + wi) % NBUF
                seen = set()
                a = wi * WTOK
                for t in range(a, min(a + WTOK, len(st))):
                    r = int(st[t])
                    if r >= TRASH or r < 0:
                        continue
                    if r in seen:
                        pc.append((buf, t, r))
                    else:
                        seen.add(r)
            pend[c] = pc
        widx0 += n_w
        if max(len(p) for p in pend) == 0:
            break
        # uniform capacity for next level
        cap = [0] * NBUF
        for c in range(w):
            cnt = [0] * NBUF
            for (b, t, r) in pend[c]:
                cnt[b] += 1
            for b in range(NBUF):
                cap[b] = max(cap[b], cnt[b])
        cap = [_pad128(x) if x else 0 for x in cap]
        lev_caps.append(cap)
        nlen = sum(cap)
        # aux base row for (level, buffer)
        for c in range(w):
            st = streams[c][lvl]
            nst = np.full(nlen, TRASH, dtype=np.int32)
            loc = [0] * NBUF
            for (b, t, r) in pend[c]:
                aux_row_local = loc[b]
                loc[b] += 1
                # aux row in buffer b: base computed later (uniform): level
                # bases = S+1 + sum of previous level caps for this buffer
                prev = sum(lc[b] for lc in lev_caps[:-1])
                st[t] = S + 1 + prev + aux_row_local
                nst[sum(cap[:b]) + aux_row_local] = r
            streams[c].append(nst)
        lvl += 1
        cur_len = nlen
        assert lvl < 12

    auxcap = sum(max(lc) for lc in lev_caps) if lev_caps else 0
    auxcap_b = [sum(lc[b] for lc in lev_caps) for b in range(NBUF)] if lev_caps \
        else [0] * NBUF
    assert S + 1 + max(auxcap_b + [0]) < 32768, auxcap_b

    def wrap16(a):
        a = np.asarray(a, np.int16)
        assert len(a) % 16 == 0
        m16 = a.reshape(-1, 16).T.copy()
        return np.tile(m16, (8, 1))

    gidx = [wrap16(g) for g in g_all]
    sidx = [wrap16(s[0]) for s in streams]
    cidx = []
    for c in range(w):
        if lvl > 0:
            cidx.append(wrap16(np.concatenate(streams[c][1:])))
        else:
            cidx.append(np.zeros((128, 8), np.int16))

    plan = dict(slot_sizes=slot_sizes, tot=tot, lev_caps=lev_caps,
                auxcap_b=auxcap_b)
    return plan, gidx, sidx, cidx


# ---------------------------------------------------------------- device program
def build_program(n, c, k, w, plan, debug=False):
    import os
    import concourse.bacc as bacc
    import concourse.mybir as mybir
    import concourse.tile as tile

    SKIP = set(os.environ.get("KSKIP", "").split(","))

    S = n // w
    C2 = 2 * c
    dt = mybir.dt
    slot_sizes = plan["slot_sizes"]
    lev_caps = plan["lev_caps"]
    tot = plan["tot"]
    max_slot = int(slot_sizes.max())
    ctot = sum(sum(lc) for lc in lev_caps)
    n_grp_tiles = WTOK // TILE

    nc = bacc.Bacc("TRN2", target_bir_lowering=False, debug=debug, num_devices=w)

    # ---- I/O ----
    xs = nc.dram_tensor("xs", [n, C2], dt.bfloat16, kind="ExternalInput")
    x_res = nc.dram_tensor("x_res", [S, c], dt.float32, kind="ExternalInput")
    r1a = nc.dram_tensor("r1a", [k, C2, c], dt.bfloat16, kind="ExternalInput")
    r1b = nc.dram_tensor("r1b", [k, C2, c], dt.bfloat16, kind="ExternalInput")
    r2a = nc.dram_tensor("r2a", [k, C2, c], dt.bfloat16, kind="ExternalInput")
    r2b = nc.dram_tensor("r2b", [k, C2, c], dt.bfloat16, kind="ExternalInput")
    b1t = nc.dram_tensor("b1t", [TILE, 8 * c], dt.float32, kind="ExternalInput")
    b2t = nc.dram_tensor("b2t", [TILE, 8 * c], dt.float32, kind="ExternalInput")
    gidx = nc.dram_tensor("gidx", [TILE, tot // 16], dt.int16, kind="ExternalInput")
    sidx = nc.dram_tensor("sidx", [TILE, tot // 16], dt.int16, kind="ExternalInput")
    cidx = nc.dram_tensor("cidx", [TILE, max(ctot, 128) // 16], dt.int16,
                          kind="ExternalInput")

    out = nc.dram_tensor("out", [S, c], dt.float32, kind="ExternalOutput")

    # scatter accumulator buffers (main S rows + trash row + aux tail)
    ZCHUNK = 3200
    rows_buf = -(-(S + 1 + max(plan["auxcap_b"] + [0]) + TILE) // ZCHUNK) * ZCHUNK
    o_bufs = [[nc.dram_tensor(f"o{ci}_{b}", [rows_buf, c], dt.float32,
                              kind="Internal")
               for b in range(NBUF)] for ci in (1, 2)]

    xs1_shard = nc.dram_tensor("xs1_shard", [S, C2], dt.bfloat16, kind="Internal")
    xs1_full = nc.dram_tensor(
        "xs1_full", [n, C2], dt.bfloat16, kind="Internal",
        addr_space="Shared" if w > 4 else "Local",
    )

    with tile.TileContext(nc) as tc:
        with (
            tc.tile_pool(name="const", bufs=1) as const_pool,
            tc.tile_pool(name="gather", bufs=4) as gpool,
            tc.tile_pool(name="gi", bufs=4) as gipool,
            tc.tile_pool(name="si", bufs=4) as sipool,
            tc.tile_pool(name="stage", bufs=6) as stpool,
            tc.tile_pool(name="psum", bufs=8, space="PSUM") as ppool,
            tc.tile_pool(name="ep", bufs=3) as eppool,
        ):
            # ---- constants ----
            wts = {}
            for name, t in (("r1a", r1a), ("r1b", r1b), ("r2a", r2a), ("r2b", r2b)):
                sb = const_pool.tile([C2, k * c], dt.bfloat16, tag=name)
                nc.sync.dma_start(
                    out=sb[:].rearrange("p (k d) -> p k d", k=k),
                    in_=t[:].rearrange("k p d -> p k d"),
                )
                wts[name] = sb
            b1_sb = const_pool.tile([TILE, 8 * c], dt.float32, tag="b1")
            nc.sync.dma_start(out=b1_sb[:], in_=b1t[:])
            b2_sb = const_pool.tile([TILE, 8 * c], dt.float32, tag="b2")
            nc.sync.dma_start(out=b2_sb[:], in_=b2t[:])

            # ---- zero accumulators ----
            zt = const_pool.tile([TILE, ZCHUNK * c // TILE], dt.float32, tag="zt")
            nc.vector.memset(zt[:], 0.0)
            for bufs in o_bufs:
                for buf in bufs:
                    for a in range(0, rows_buf, ZCHUNK):
                        nc.sync.dma_start(
                            out=buf[a: a + ZCHUNK, :].rearrange(
                                "(p g) d -> p (g d)", p=TILE),
                            in_=zt[:],
                        )

            # ---- one sparse conv pass ----
            def conv(src_table, ra_sb, rb_sb, bufs):
                tt = 0
                widx = 0
                stage_t = None
                psum_t = None

                def flush_group(n_tiles):
                    nonlocal widx
                    if "scat" in SKIP:
                        widx += 1
                        return
                    ntok = n_tiles * TILE
                    base16 = (tt - n_tiles) * (TILE // 16)
                    si_t = sipool.tile([TILE, WTOK // 16], dt.int16, tag="si")
                    nc.sync.dma_start(
                        out=si_t[:, : ntok // 16],
                        in_=sidx[:, base16: base16 + ntok // 16],
                    )
                    nc.gpsimd.dma_scatter_add(
                        bufs[widx % NBUF][:],
                        stage_t[:, : ntok * c // TILE].rearrange(
                            "p (g d) -> p g d", d=c),
                        si_t[:, : ntok // 16],
                        ntok, ntok, c,
                    )
                    widx += 1

                for kk in range(k):
                    for j in range(w):
                        L = int(slot_sizes[kk, j])
                        g_t = gpool.tile([TILE, 1, max_slot], dt.bfloat16, tag="g")
                        if "gath" in SKIP:
                            nc.vector.memset(g_t[:, 0, :L], 0)
                        else:
                            gi_t = gipool.tile([TILE, max_slot // 16], dt.int16,
                                               tag="gi")
                            base16 = tt * (TILE // 16)
                            nc.sync.dma_start(
                                out=gi_t[:, : L // 16],
                                in_=gidx[:, base16: base16 + L // 16],
                            )
                            nc.gpsimd.dma_gather(
                                g_t[:, :, :L],
                                src_table[j * S: (j + 1) * S, :],
                                gi_t[:, : L // 16],
                                L, L, C2,
                                transpose=True,
                                single_packet=False,
                            )
                        for t in range(L // TILE):
                            b = tt % n_grp_tiles
                            if b == 0:
                                stage_t = stpool.tile(
                                    [TILE, WTOK * c // TILE], dt.float32, tag="st")
                            if b % 8 == 0:
                                psum_t = ppool.tile([TILE, 512], dt.float32,
                                                    tag="ps")
                            ps = psum_t[:, (b % 8) * c: (b % 8 + 1) * c]
                            lhsT = g_t[:, 0, t * TILE: (t + 1) * TILE]
                            nc.tensor.matmul(
                                out=ps, lhsT=lhsT,
                                rhs=ra_sb[:, kk * c: (kk + 1) * c],
                                start=True, stop=False)
                            nc.tensor.matmul(
                                out=ps, lhsT=lhsT,
                                rhs=rb_sb[:, kk * c: (kk + 1) * c],
                                start=False, stop=True)
                            if b % 8 == 7:
                                nc.vector.tensor_copy(
                                    out=stage_t[:, (b - 7) * c: (b + 1) * c],
                                    in_=psum_t[:])
                            elif b == n_grp_tiles - 1:
                                nb = b % 8 + 1
                                nc.vector.tensor_copy(
                                    out=stage_t[:, (b + 1 - nb) * c: (b + 1) * c],
                                    in_=psum_t[:, : nb * c])
                            tt += 1
                            if tt % n_grp_tiles == 0:
                                flush_group(n_grp_tiles)
                rem = tt % n_grp_tiles
                if rem:
                    full_banks = rem // 8
                    tail = rem % 8
                    if tail:
                        nc.vector.tensor_copy(
                            out=stage_t[:, full_banks * 8 * c: rem * c],
                            in_=psum_t[:, : tail * c])
                    flush_group(rem)

                # ---- cleanup levels: fold aux rows back ----
                coff16 = 0     # offset into cidx (16ths)
                for li, cap in enumerate(lev_caps if "cleanup" not in SKIP else []):
                    # aux base row for this level per buffer
                    lev_base = [S + 1 + sum(lc[bb] for lc in lev_caps[:li])
                                for bb in range(NBUF)]
                    stream_len = sum(cap)
                    n_w = (stream_len + WTOK - 1) // WTOK
                    # read segments: buffer bb occupies stream positions
                    # [sum(cap[:bb]), +cap[bb])
                    for wi in range(n_w):
                        a = wi * WTOK
                        e = min(a + WTOK, stream_len)
                        ntok = e - a
                        st = stpool.tile([TILE, WTOK * c // TILE], dt.float32,
                                         tag="st")
                        # DMA the pieces of [a, e) from their buffers
                        for bb in range(NBUF):
                            sb0, sb1 = sum(cap[:bb]), sum(cap[:bb + 1])
                            ov0, ov1 = max(a, sb0), min(e, sb1)
                            if ov0 >= ov1:
                                continue
                            rows0 = lev_base[bb] + (ov0 - sb0)
                            cnt = ov1 - ov0
                            nc.sync.dma_start(
                                out=st[:].rearrange("p (g d) -> p g d", d=c)
                                [:, (ov0 - a) // TILE: (ov1 - a) // TILE, :],
                                in_=bufs[bb][rows0: rows0 + cnt, :]
                                .rearrange("(g p) d -> p g d", p=TILE),
                            )
                        si_t = sipool.tile([TILE, WTOK // 16], dt.int16, tag="si")
                        nc.sync.dma_start(
                            out=si_t[:, : ntok // 16],
                            in_=cidx[:, coff16 + a // 16: coff16 + e // 16],
                        )
                        nc.gpsimd.dma_scatter_add(
                            bufs[widx % NBUF][:],
                            st[:, : ntok * c // TILE].rearrange(
                                "p (g d) -> p g d", d=c),
                            si_t[:, : ntok // 16],
                            ntok, ntok, c,
                        )
                        widx += 1
                    coff16 += stream_len // 16

            # ======== conv1 ========
            if "conv1" not in SKIP:
                conv(xs, wts["r1a"], wts["r1b"], o_bufs[0])

            # ======== epilogue1: sum buffers + bias + relu + split ========
            EPR = min(1024, S)  # rows per epilogue tile
            G = EPR // TILE
            n_ep = -(-S // EPR)
            for i in range(n_ep):
                r0 = min(i * EPR, S - EPR)
                acc = eppool.tile([TILE, G, c], dt.float32, tag="ea")
                tmp = eppool.tile([TILE, G, c], dt.float32, tag="eb")
                for b in range(NBUF):
                    dst = acc if b == 0 else tmp
                    nc.sync.dma_start(
                        out=dst[:],
                        in_=o_bufs[0][b][r0: r0 + EPR, :].rearrange(
                            "(g p) d -> p g d", p=TILE))
                    if b > 0:
                        nc.vector.tensor_add(out=acc[:], in0=acc[:], in1=tmp[:])
                b1v = b1_sb[:].rearrange("p (g d) -> p g d", d=c)[:, :G, :]
                nc.vector.tensor_add(out=acc[:], in0=acc[:], in1=b1v)
                nc.vector.tensor_scalar_max(acc[:], acc[:], 0.0)
                pack = eppool.tile([TILE, G, C2], dt.bfloat16, tag="ep")
                nc.vector.tensor_copy(out=pack[:, :, :c], in_=acc[:])  # hi
                hif = eppool.tile([TILE, G, c], dt.float32, tag="eh")
                nc.vector.tensor_copy(out=hif[:], in_=pack[:, :, :c])
                nc.vector.tensor_sub(out=acc[:], in0=acc[:], in1=hif[:])
                nc.vector.tensor_copy(out=pack[:, :, c:], in_=acc[:])  # lo
                nc.sync.dma_start(
                    out=xs1_shard[r0: r0 + EPR, :].rearrange(
                        "(g p) d -> p g d", p=TILE),
                    in_=pack[:])

            # ======== allgather ========
            if w > 1 and "cc" not in SKIP:
                nc.gpsimd.collective_compute(
                    "AllGather", mybir.AluOpType.bypass,
                    replica_groups=[list(range(w))],
                    ins=[xs1_shard[:]], outs=[xs1_full[:]])
                conv2_src = xs1_full
            else:
                conv2_src = xs1_shard

            # ======== conv2 ========
            if "conv2" not in SKIP:
                conv(conv2_src, wts["r2a"], wts["r2b"], o_bufs[1])

            # ======== epilogue2: sum buffers + bias + residual + relu ========
            for i in range(n_ep):
                r0 = min(i * EPR, S - EPR)
                acc = eppool.tile([TILE, G, c], dt.float32, tag="ea")
                tmp = eppool.tile([TILE, G, c], dt.float32, tag="eb")
                for b in range(NBUF):
                    dst = acc if b == 0 else tmp
                    nc.sync.dma_start(
                        out=dst[:],
                        in_=o_bufs[1][b][r0: r0 + EPR, :].rearrange(
                            "(g p) d -> p g d", p=TILE))
                    if b > 0:
                        nc.vector.tensor_add(out=acc[:], in0=acc[:], in1=tmp[:])
                b2v = b2_sb[:].rearrange("p (g d) -> p g d", d=c)[:, :G, :]
                nc.vector.tensor_add(out=acc[:], in0=acc[:], in1=b2v)
                xr = eppool.tile([TILE, G, c], dt.float32, tag="ex")
                nc.sync.dma_start(
                    out=xr[:],
                    in_=x_res[r0: r0 + EPR, :].rearrange("(g p) d -> p g d",
                                                         p=TILE))
                nc.vector.tensor_add(out=acc[:], in0=acc[:], in1=xr[:])
                nc.vector.tensor_scalar_max(acc[:], acc[:], 0.0)
                nc.sync.dma_start(
                    out=out[r0: r0 + EPR, :].rearrange("(g p) d -> p g d",
                                                       p=TILE),
                    in_=acc[:])

    nc.compile()
    return nc


# ---------------------------------------------------------------- host wrapper
def prepare(x, w1, w2, gamma1, beta1, mean1, var1, gamma2, beta2, mean2, var2,
            in_map, out_map, n=N, w=W):
    x = np.asarray(x, np.float32)
    s1 = (np.asarray(gamma1, np.float32)
          / np.sqrt(np.asarray(var1, np.float32) + EPS))
    b1 = np.asarray(beta1, np.float32) - np.asarray(mean1, np.float32) * s1
    s2 = (np.asarray(gamma2, np.float32)
          / np.sqrt(np.asarray(var2, np.float32) + EPS))
    b2 = np.asarray(beta2, np.float32) - np.asarray(mean2, np.float32) * s2

    r1a, r1b = _weight_stacks(np.asarray(w1, np.float32) * s1[None, None, :])
    r2a, r2b = _weight_stacks(np.asarray(w2, np.float32) * s2[None, None, :])

    xs = _make_split_table(x)
    b1_tile = np.tile(b1[None, :], (TILE, 8)).astype(np.float32)
    b2_tile = np.tile(b2[None, :], (TILE, 8)).astype(np.float32)

    plan, gidx_all, sidx_all, cidx_all = _prep_indices_static(
        np.asarray(in_map), np.asarray(out_map), n, w)

    S = n // w
    in_maps = []
    for c in range(w):
        in_maps.append(dict(
            xs=np.ascontiguousarray(xs),
            x_res=np.ascontiguousarray(x[c * S:(c + 1) * S]),
            r1a=r1a, r1b=r1b, r2a=r2a, r2b=r2b,
            b1t=b1_tile, b2t=b2_tile,
            gidx=np.ascontiguousarray(gidx_all[c]),
            sidx=np.ascontiguousarray(sidx_all[c]),
            cidx=np.ascontiguousarray(cidx_all[c]),
        ))
    return plan, in_maps


def kernel(**inputs):
    from concourse import bass_utils

    plan, in_maps = prepare(**inputs)
    nc = build_program(N, C, K, W, plan)
    res = bass_utils.run_bass_kernel_spmd(nc, in_maps, core_ids=list(range(W)))
    S = N // W
    out = np.concatenate([res.results[c]["out"][:S] for c in range(W)], axis=0)
    return out.astype(np.float32)



# revision 18
# speedup vs baseline: 1.4323x; 1.4323x over previous
"""Trainium2 Bass kernel for a MinkowskiNet BasicBlock:
    out = relu(bn2(conv(relu(bn1(conv(x, w1))), w2)) + x)
with gather-GEMM-scatter sparse convolutions over (in_map, out_map) pair lists.

Strategy (8 NeuronCores, SPMD):
  - Shard by output-voxel owner: core c owns output rows [c*S, (c+1)*S), S = N/8.
  - Replicate x (f32 [N, 64], 256B rows) and weights to all cores.
  - Gather rows with dma_gather(transpose=False): tokens land on partitions
    ([128 tok, L/128, 64] f32).  Non-transpose gathers avoid the xbar, so
    they can be spread across all 4 SWDGE queues (4 Q7 core pairs generate
    descriptors in parallel; queue = DMASW-lane % 4 keeps Tile's cumulative
    lane-semaphore accounting FIFO within each lane).
  - Per 2-tile group: DVE cast f32->bf16, then one HWDGE dma_start_transpose
    [128 tok, 128] -> [128 (chA|chB), 128 tok] feeds the matmul lhsT.  The
    xbar has a single user (HWDGE transposes, FIFO) so no corruption.
  - One bf16 matmul per 128-token tile: rhs is [w;0] / [0;w] stacked per
    kernel offset, PSUM f32 accumulate, 8 tiles per PSUM bank.
  - Scatter-add with dma_scatter_add (CCE f32), also spread over 4 queues.
    Windows of 2048 tokens rotate over B=4 accumulator buffers; in-window
    duplicate rows are redirected to aux rows folded back by host-planned
    cleanup scatter passes.
  - BN folded: scale into weights (host side), bias added post-scatter.
  - Intermediate activation (f32) AllGather'd across cores.
"""

import sys

if "/opt/trn_rl_repo" not in sys.path:
    sys.path.insert(0, "/opt/trn_rl_repo")

import numpy as np
import ml_dtypes

BF16 = ml_dtypes.bfloat16

# ---------------------------------------------------------------- problem cfg
N = 200000  # voxels
C = 64      # channels
K = 27      # kernel offsets
M = 100000  # pairs per offset
W = 8       # cores
EPS = 1e-5

WTOK = 2048   # tokens per scatter window (one dma_scatter_add call)
TILE = 128    # tokens per matmul tile
PAIR = 256    # tokens per xbar transpose block (2 tiles)
NBUF = 4      # rotating scatter accumulator buffers


# ---------------------------------------------------------------- host-side prep
def _pad128(n):
    return ((int(n) + 127) // 128) * 128


def _pad256(n):
    return ((int(n) + 255) // 256) * 256


def _weight_stacks(w_scaled):
    """[K, C, C] f32 -> [K, 2, 2C, C] bf16 with [w;0] and [0;w] stacks."""
    k, c, _ = w_scaled.shape
    wb = w_scaled.astype(BF16)
    out = np.zeros((k, 2, 2 * c, c), dtype=BF16)
    out[:, 0, :c, :] = wb
    out[:, 1, c:, :] = wb
    return np.ascontiguousarray(out)


def _prep_indices_static(in_map, out_map, n, w):
    """Deterministic two-pass version: aux rows laid out per (level, buffer)
    with uniform capacities so the device program is core-independent."""
    S = n // w
    kk = in_map.shape[0]
    TRASH = S

    owner = out_map // S
    chunk = in_map // S
    counts = np.zeros((w, kk, w), dtype=np.int64)
    for k in range(kk):
        flat = owner[k] * w + chunk[k]
        counts[:, k, :] = np.bincount(flat, minlength=w * w).reshape(w, w)
    slot_sizes = np.maximum(((counts.max(axis=0) + 255) // 256) * 256, 256)
    tot = int(slot_sizes.sum())
    n_win = (tot + WTOK - 1) // WTOK

    g_all, s_raw = [], []
    for c in range(w):
        g_stream = np.zeros(tot, dtype=np.int32)
        s_stream = np.full(tot, TRASH, dtype=np.int32)
        off = 0
        for k in range(kk):
            sel_c = owner[k] == c
            i_k = in_map[k][sel_c]
            o_k = out_map[k][sel_c] - c * S
            ch_k = chunk[k][sel_c]
            for j in range(w):
                L = int(slot_sizes[k, j])
                selj = ch_k == j
                i_loc = i_k[selj] - j * S
                o_loc = o_k[selj]
                order = np.argsort(o_loc, kind="stable")
                cnt = len(i_loc)
                g_stream[off:off + cnt] = i_loc[order]
                s_stream[off:off + cnt] = o_loc[order]
                off += L
        g_all.append(g_stream)
        s_raw.append(s_stream)

    # ---- iterative dedup with per-level uniform capacities ----
    # level 0 = main stream; dups of level l become level l+1 tokens.
    streams = [[s] for s in s_raw]             # per core: [lvl0, lvl1, ...]
    pend = [None] * w                          # per core: list[(buf, true_r)]
    lev_caps = []                              # per level: [cap_b] * NBUF
    widx0 = 0
    lvl = 0
    cur_len = tot
    while True:
        n_w = (cur_len + WTOK - 1) // WTOK
        for c in range(w):
            st = streams[c][lvl]
            pc = []
            for wi in range(n_w):
                buf = (widx0 + wi) % NBUF
                seen = set()
                a = wi * WTOK
                for t in range(a, min(a + WTOK, len(st))):
                    r = int(st[t])
                    if r >= TRASH or r < 0:
                        continue
                    if r in seen:
                        pc.append((buf, t, r))
                    else:
                        seen.add(r)
            pend[c] = pc
        widx0 += n_w
        if max(len(p) for p in pend) == 0:
            break
        cap = [0] * NBUF
        for c in range(w):
            cnt = [0] * NBUF
            for (b, t, r) in pend[c]:
                cnt[b] += 1
            for b in range(NBUF):
                cap[b] = max(cap[b], cnt[b])
        cap = [_pad128(x) if x else 0 for x in cap]
        lev_caps.append(cap)
        nlen = sum(cap)
        for c in range(w):
            st = streams[c][lvl]
            nst = np.full(nlen, TRASH, dtype=np.int32)
            loc = [0] * NBUF
            for (b, t, r) in pend[c]:
                aux_row_local = loc[b]
                loc[b] += 1
                prev = sum(lc[b] for lc in lev_caps[:-1])
                st[t] = S + 1 + prev + aux_row_local
                nst[sum(cap[:b]) + aux_row_local] = r
            streams[c].append(nst)
        lvl += 1
        cur_len = nlen
        assert lvl < 12

    auxcap_b = [sum(lc[b] for lc in lev_caps) for b in range(NBUF)] if lev_caps \
        else [0] * NBUF
    assert S + 1 + max(auxcap_b + [0]) < 32768, auxcap_b

    def wrap16(a):
        a = np.asarray(a, np.int16)
        assert len(a) % 16 == 0
        m16 = a.reshape(-1, 16).T.copy()
        return np.tile(m16, (8, 1))

    gidx = [wrap16(g) for g in g_all]
    sidx = [wrap16(s[0]) for s in streams]
    cidx = []
    for c in range(w):
        if lvl > 0:
            cidx.append(wrap16(np.concatenate(streams[c][1:])))
        else:
            cidx.append(np.zeros((128, 8), np.int16))

    plan = dict(slot_sizes=slot_sizes, tot=tot, lev_caps=lev_caps,
                auxcap_b=auxcap_b)
    return plan, gidx, sidx, cidx


# ---------------------------------------------------------------- device program
def build_program(n, c, k, w, plan, debug=False):
    import os
    import concourse.bacc as bacc
    import concourse.mybir as mybir
    import concourse.tile as tile

    SKIP = set(os.environ.get("KSKIP", "").split(","))

    S = n // w
    C2 = 2 * c
    dt = mybir.dt
    slot_sizes = plan["slot_sizes"]
    lev_caps = plan["lev_caps"]
    tot = plan["tot"]
    max_slot = int(slot_sizes.max())
    ctot = sum(sum(lc) for lc in lev_caps)
    n_grp_tiles = WTOK // TILE

    nc = bacc.Bacc("TRN2", target_bir_lowering=False, debug=debug, num_devices=w,
                   num_swdge_queues=4)

    # ---- I/O ----
    xs = nc.dram_tensor("xs", [n, c], dt.float32, kind="ExternalInput")
    ident = nc.dram_tensor("ident", [TILE, TILE], dt.bfloat16,
                           kind="ExternalInput")
    x_res = nc.dram_tensor("x_res", [S, c], dt.float32, kind="ExternalInput")
    r1 = nc.dram_tensor("r1", [k, 2, C2, c], dt.bfloat16, kind="ExternalInput")
    r2 = nc.dram_tensor("r2", [k, 2, C2, c], dt.bfloat16, kind="ExternalInput")
    b1t = nc.dram_tensor("b1t", [TILE, 8 * c], dt.float32, kind="ExternalInput")
    b2t = nc.dram_tensor("b2t", [TILE, 8 * c], dt.float32, kind="ExternalInput")
    gidx = nc.dram_tensor("gidx", [TILE, tot // 16], dt.int16, kind="ExternalInput")
    sidx = nc.dram_tensor("sidx", [TILE, tot // 16], dt.int16, kind="ExternalInput")
    cidx = nc.dram_tensor("cidx", [TILE, max(ctot, 128) // 16], dt.int16,
                          kind="ExternalInput")

    out = nc.dram_tensor("out", [S, c], dt.float32, kind="ExternalOutput")

    # scatter accumulator buffers (main S rows + trash row + aux tail)
    ZCHUNK = 3200
    rows_buf = -(-(S + 1 + max(plan["auxcap_b"] + [0]) + TILE) // ZCHUNK) * ZCHUNK
    o_bufs = [[nc.dram_tensor(f"o{ci}_{b}", [rows_buf, c], dt.float32,
                              kind="Internal")
               for b in range(NBUF)] for ci in (1, 2)]

    xs1_shard = nc.dram_tensor("xs1_shard", [S, c], dt.float32, kind="Internal")
    xs1_full = nc.dram_tensor(
        "xs1_full", [n, c], dt.float32, kind="Internal",
        addr_space="Shared" if w > 4 else "Local",
    )

    with tile.TileContext(nc) as tc:
        with (
            tc.tile_pool(name="const", bufs=1) as const_pool,
            tc.tile_pool(name="gather", bufs=4) as gpool,
            tc.tile_pool(name="gcast", bufs=3) as gcpool,
            tc.tile_pool(name="gtra", bufs=3) as gtpool,
            tc.tile_pool(name="gi", bufs=4) as gipool,
            tc.tile_pool(name="si", bufs=4) as sipool,
            tc.tile_pool(name="stage", bufs=6) as stpool,
            tc.tile_pool(name="psum", bufs=4, space="PSUM") as ppool,
            tc.tile_pool(name="ptp", bufs=4, space="PSUM") as tppool,
            tc.tile_pool(name="ep", bufs=3) as eppool,
        ):
            # ---- constants ----
            wts = {}
            for name, t in (("r1", r1), ("r2", r2)):
                sb = const_pool.tile([C2, k * 2 * c], dt.bfloat16, tag=name)
                nc.sync.dma_start(
                    out=sb[:].rearrange("p (k h d) -> p k h d", k=k, h=2),
                    in_=t[:].rearrange("k h p d -> p k h d"),
                )
                wts[name] = sb
            b1_sb = const_pool.tile([TILE, 8 * c], dt.float32, tag="b1")
            nc.sync.dma_start(out=b1_sb[:], in_=b1t[:])
            b2_sb = const_pool.tile([TILE, 8 * c], dt.float32, tag="b2")
            nc.sync.dma_start(out=b2_sb[:], in_=b2t[:])
            id_sb = const_pool.tile([TILE, TILE], dt.bfloat16, tag="id")
            nc.sync.dma_start(out=id_sb[:], in_=ident[:])

            # ---- zero accumulators ----
            zt = const_pool.tile([TILE, ZCHUNK * c // TILE], dt.float32, tag="zt")
            nc.vector.memset(zt[:], 0.0)
            for bufs in o_bufs:
                for buf in bufs:
                    for a in range(0, rows_buf, ZCHUNK):
                        nc.sync.dma_start(
                            out=buf[a: a + ZCHUNK, :].rearrange(
                                "(p g) d -> p (g d)", p=TILE),
                            in_=zt[:],
                        )

            # ---- one sparse conv pass ----
            def conv(src_table, w_sb, bufs):
                tt = 0
                widx = 0
                stage_t = None
                psum_t = None

                def flush_group(n_tiles):
                    nonlocal widx
                    if "scat" in SKIP:
                        widx += 1
                        return
                    ntok = n_tiles * TILE
                    base16 = (tt - n_tiles) * (TILE // 16)
                    si_t = sipool.tile([TILE, WTOK // 16], dt.int16, tag="si")
                    nc.sync.dma_start(
                        out=si_t[:, : ntok // 16],
                        in_=sidx[:, base16: base16 + ntok // 16],
                    )
                    nc.gpsimd.dma_scatter_add(
                        bufs[widx % NBUF][:],
                        stage_t[:, : ntok * c // TILE].rearrange(
                            "p (g d) -> p g d", d=c),
                        si_t[:, : ntok // 16],
                        ntok, ntok, c,
                    )
                    widx += 1

                for kk in range(k):
                    for j in range(w):
                        L = int(slot_sizes[kk, j])
                        g_t = gpool.tile([TILE, max_slot // TILE, c],
                                         dt.float32, tag="g")
                        gb_t = gcpool.tile([TILE, max_slot // TILE, c],
                                           dt.bfloat16, tag="gb")
                        gT_t = gtpool.tile([TILE, max_slot // PAIR, TILE],
                                           dt.bfloat16, tag="gT")
                        if "gath" in SKIP:
                            nc.vector.memset(g_t[:, : L // TILE, :], 0)
                        else:
                            gi_t = gipool.tile([TILE, max_slot // 16], dt.int16,
                                               tag="gi")
                            base16 = tt * (TILE // 16)
                            nc.sync.dma_start(
                                out=gi_t[:, : L // 16],
                                in_=gidx[:, base16: base16 + L // 16],
                            )
                            nc.gpsimd.dma_gather(
                                g_t[:, : L // TILE, :],
                                src_table[j * S: (j + 1) * S, :],
                                gi_t[:, : L // 16],
                                L, L, c,
                                transpose=False,
                                single_packet=False,
                            )
                        # cast f32 -> bf16 (bulk, one ACT op per slot)
                        nc.scalar.copy(out=gb_t[:, : L // TILE, :],
                                       in_=g_t[:, : L // TILE, :])
                        # PE transpose per 256-token pair:
                        # [128 tok, 128 (2x64ch)] -> [128 (chA|chB), 128 tok]
                        for q in range(L // PAIR):
                            tp = tppool.tile([TILE, TILE], dt.bfloat16,
                                             tag="tp")
                            nc.tensor.transpose(
                                out=tp[:],
                                in_=gb_t[:, 2 * q: 2 * q + 2, :],
                                identity=id_sb[:])
                            nc.vector.tensor_copy(out=gT_t[:, q, :],
                                                  in_=tp[:])
                        for t in range(L // TILE):
                            b = tt % n_grp_tiles
                            if b == 0:
                                stage_t = stpool.tile(
                                    [TILE, WTOK * c // TILE], dt.float32, tag="st")
                            if b % 8 == 0:
                                psum_t = ppool.tile([TILE, 512], dt.float32,
                                                    tag="ps")
                            ps = psum_t[:, (b % 8) * c: (b % 8 + 1) * c]
                            nc.tensor.matmul(
                                out=ps,
                                lhsT=gT_t[:, t // 2, :],
                                rhs=w_sb[:, (kk * 2 + t % 2) * c:
                                         (kk * 2 + t % 2 + 1) * c],
                                start=True, stop=True)
                            if b % 8 == 7:
                                nc.vector.tensor_copy(
                                    out=stage_t[:, (b - 7) * c: (b + 1) * c],
                                    in_=psum_t[:])
                            elif b == n_grp_tiles - 1:
                                nb = b % 8 + 1
                                nc.vector.tensor_copy(
                                    out=stage_t[:, (b + 1 - nb) * c: (b + 1) * c],
                                    in_=psum_t[:, : nb * c])
                            tt += 1
                            if tt % n_grp_tiles == 0:
                                flush_group(n_grp_tiles)
                rem = tt % n_grp_tiles
                if rem:
                    full_banks = rem // 8
                    tail = rem % 8
                    if tail:
                        nc.vector.tensor_copy(
                            out=stage_t[:, full_banks * 8 * c: rem * c],
                            in_=psum_t[:, : tail * c])
                    flush_group(rem)

                # ---- cleanup levels: fold aux rows back ----
                coff16 = 0     # offset into cidx (16ths)
                for li, cap in enumerate(lev_caps if "cleanup" not in SKIP else []):
                    lev_base = [S + 1 + sum(lc[bb] for lc in lev_caps[:li])
                                for bb in range(NBUF)]
                    stream_len = sum(cap)
                    n_w = (stream_len + WTOK - 1) // WTOK
                    for wi in range(n_w):
                        a = wi * WTOK
                        e = min(a + WTOK, stream_len)
                        ntok = e - a
                        st = stpool.tile([TILE, WTOK * c // TILE], dt.float32,
                                         tag="st")
                        for bb in range(NBUF):
                            sb0, sb1 = sum(cap[:bb]), sum(cap[:bb + 1])
                            ov0, ov1 = max(a, sb0), min(e, sb1)
                            if ov0 >= ov1:
                                continue
                            rows0 = lev_base[bb] + (ov0 - sb0)
                            cnt = ov1 - ov0
                            nc.sync.dma_start(
                                out=st[:].rearrange("p (g d) -> p g d", d=c)
                                [:, (ov0 - a) // TILE: (ov1 - a) // TILE, :],
                                in_=bufs[bb][rows0: rows0 + cnt, :]
                                .rearrange("(g p) d -> p g d", p=TILE),
                            )
                        si_t = sipool.tile([TILE, WTOK // 16], dt.int16, tag="si")
                        nc.sync.dma_start(
                            out=si_t[:, : ntok // 16],
                            in_=cidx[:, coff16 + a // 16: coff16 + e // 16],
                        )
                        nc.gpsimd.dma_scatter_add(
                            bufs[widx % NBUF][:],
                            st[:, : ntok * c // TILE].rearrange(
                                "p (g d) -> p g d", d=c),
                            si_t[:, : ntok // 16],
                            ntok, ntok, c,
                        )
                        widx += 1
                    coff16 += stream_len // 16

            # ======== conv1 ========
            if "conv1" not in SKIP:
                conv(xs, wts["r1"], o_bufs[0])

            # ======== epilogue1: sum buffers + bias + relu ========
            EPR = min(1024, S)  # rows per epilogue tile
            G = EPR // TILE
            n_ep = -(-S // EPR)
            for i in range(n_ep):
                r0 = min(i * EPR, S - EPR)
                acc = eppool.tile([TILE, G, c], dt.float32, tag="ea")
                tmp = eppool.tile([TILE, G, c], dt.float32, tag="eb")
                for b in range(NBUF):
                    dst = acc if b == 0 else tmp
                    nc.sync.dma_start(
                        out=dst[:],
                        in_=o_bufs[0][b][r0: r0 + EPR, :].rearrange(
                            "(g p) d -> p g d", p=TILE))
                    if b > 0:
                        nc.vector.tensor_add(out=acc[:], in0=acc[:], in1=tmp[:])
                b1v = b1_sb[:].rearrange("p (g d) -> p g d", d=c)[:, :G, :]
                nc.vector.tensor_add(out=acc[:], in0=acc[:], in1=b1v)
                nc.vector.tensor_scalar_max(acc[:], acc[:], 0.0)
                nc.sync.dma_start(
                    out=xs1_shard[r0: r0 + EPR, :].rearrange(
                        "(g p) d -> p g d", p=TILE),
                    in_=acc[:])

            # ======== allgather ========
            if w > 1 and "cc" not in SKIP:
                nc.gpsimd.collective_compute(
                    "AllGather", mybir.AluOpType.bypass,
                    replica_groups=[list(range(w))],
                    ins=[xs1_shard[:]], outs=[xs1_full[:]])
                conv2_src = xs1_full
            else:
                conv2_src = xs1_shard

            # ======== conv2 ========
            if "conv2" not in SKIP:
                conv(conv2_src, wts["r2"], o_bufs[1])

            # ======== epilogue2: sum buffers + bias + residual + relu ========
            for i in range(n_ep):
                r0 = min(i * EPR, S - EPR)
                acc = eppool.tile([TILE, G, c], dt.float32, tag="ea")
                tmp = eppool.tile([TILE, G, c], dt.float32, tag="eb")
                for b in range(NBUF):
                    dst = acc if b == 0 else tmp
                    nc.sync.dma_start(
                        out=dst[:],
                        in_=o_bufs[1][b][r0: r0 + EPR, :].rearrange(
                            "(g p) d -> p g d", p=TILE))
                    if b > 0:
                        nc.vector.tensor_add(out=acc[:], in0=acc[:], in1=tmp[:])
                b2v = b2_sb[:].rearrange("p (g d) -> p g d", d=c)[:, :G, :]
                nc.vector.tensor_add(out=acc[:], in0=acc[:], in1=b2v)
                xr = eppool.tile([TILE, G, c], dt.float32, tag="ex")
                nc.sync.dma_start(
                    out=xr[:],
                    in_=x_res[r0: r0 + EPR, :].rearrange("(g p) d -> p g d",
                                                         p=TILE))
                nc.vector.tensor_add(out=acc[:], in0=acc[:], in1=xr[:])
                nc.vector.tensor_scalar_max(acc[:], acc[:], 0.0)
                nc.sync.dma_start(
                    out=out[r0: r0 + EPR, :].rearrange("(g p) d -> p g d",
                                                       p=TILE),
                    in_=acc[:])

    # ---- spread SWDGE desc-gen across the 4 Q7 core pairs --------------
    # Each dma_gather/dma_scatter_add is serviced by Q7 core pair
    # `queue_num` (ucode: cpu_id/2 == queue_num), so 4 queues give 4x
    # descriptor-generation throughput.  Correctness constraints: (1) the
    # xbar is a single shared resource, so only non-transpose gathers are
    # safe to spread; (2) Tile's DMASW lane semaphores count completions
    # cumulatively, which assumes FIFO completion within a lane — queue =
    # lane % 4 keeps every lane single-queue, and a queue's ring drains
    # (and fires sems) in push order, so per-lane FIFO still holds.
    from concourse.tile_sem_assignment import PROC_NAME_TO_IDX

    QMODE = os.environ.get("QMODE", "lane")
    dmasw_lane = {PROC_NAME_TO_IDX[f"DMASW{i}"]: i for i in range(8)}
    for f in nc.m.functions:
        for blk in f.blocks:
            for inst in blk.instructions:
                if not isinstance(inst, (mybir.InstDMAGatherAnt,
                                         mybir.InstDMAScatterAddAnt)):
                    continue
                lane = dmasw_lane.get(getattr(inst, "bass_scheduled_proc", None))
                if lane is None:
                    continue
                if QMODE == "lane":
                    inst.queue_num = lane % 4
                elif QMODE == "q01":
                    inst.queue_num = lane % 2
                # QMODE == "off": leave queue 0 everywhere

    nc.compile()
    return nc


# ---------------------------------------------------------------- host wrapper
def prepare(x, w1, w2, gamma1, beta1, mean1, var1, gamma2, beta2, mean2, var2,
            in_map, out_map, n=N, w=W):
    x = np.asarray(x, np.float32)
    s1 = (np.asarray(gamma1, np.float32)
          / np.sqrt(np.asarray(var1, np.float32) + EPS))
    b1 = np.asarray(beta1, np.float32) - np.asarray(mean1, np.float32) * s1
    s2 = (np.asarray(gamma2, np.float32)
          / np.sqrt(np.asarray(var2, np.float32) + EPS))
    b2 = np.asarray(beta2, np.float32) - np.asarray(mean2, np.float32) * s2

    r1 = _weight_stacks(np.asarray(w1, np.float32) * s1[None, None, :])
    r2 = _weight_stacks(np.asarray(w2, np.float32) * s2[None, None, :])

    b1_tile = np.tile(b1[None, :], (TILE, 8)).astype(np.float32)
    b2_tile = np.tile(b2[None, :], (TILE, 8)).astype(np.float32)

    plan, gidx_all, sidx_all, cidx_all = _prep_indices_static(
        np.asarray(in_map), np.asarray(out_map), n, w)

    S = n // w
    in_maps = []
    for c in range(w):
        in_maps.append(dict(
            xs=np.ascontiguousarray(x),
            ident=np.eye(TILE, dtype=BF16),
            x_res=np.ascontiguousarray(x[c * S:(c + 1) * S]),
            r1=r1, r2=r2,
            b1t=b1_tile, b2t=b2_tile,
            gidx=np.ascontiguousarray(gidx_all[c]),
            sidx=np.ascontiguousarray(sidx_all[c]),
            cidx=np.ascontiguousarray(cidx_all[c]),
        ))
    return plan, in_maps


def kernel(**inputs):
    from concourse import bass_utils

    plan, in_maps = prepare(**inputs)
    nc = build_program(N, C, K, W, plan)
    res = bass_utils.run_bass_kernel_spmd(nc, in_maps, core_ids=list(range(W)))
    S = N // W
    out = np.concatenate([res.results[c]["out"][:S] for c in range(W)], axis=0)
    return out.astype(np.float32)


# revision 35
# speedup vs baseline: 1.6832x; 1.1752x over previous
"""Trainium2 Bass kernel for a MinkowskiNet BasicBlock:
    out = relu(bn2(conv(relu(bn1(conv(x, w1))), w2)) + x)
with gather-GEMM-scatter sparse convolutions over (in_map, out_map) pair lists.

Strategy (8 NeuronCores, SPMD):
  - Shard by output-voxel owner: core c owns output rows [c*S, (c+1)*S), S = N/8.
  - Replicate x (f32 [N, 64], 256B rows) and weights to all cores.
  - Gather rows with dma_gather(transpose=False): tokens land on partitions
    ([128 tok, L/128, 64] f32).  Non-transpose gathers avoid the xbar, so
    they can be spread across all 4 SWDGE queues (4 Q7 core pairs generate
    descriptors in parallel; queue = DMASW-lane % 4 keeps Tile's cumulative
    lane-semaphore accounting FIFO within each lane).
  - Per 2-tile group: ACT cast f32->bf16, then a PE transpose (matmul with
    identity) [128 tok, 128] -> [128 (chA|chB), 128 tok] feeds the GEMM
    lhsT.  (The DMA xbar is avoided entirely: it is a single shared
    resource and corrupts under multi-queue concurrency.)
  - One bf16 matmul per 128-token tile: rhs is [w;0] / [0;w] stacked per
    kernel offset, PSUM f32 accumulate, 8 tiles per PSUM bank.
  - Scatter-add with dma_scatter_add into SBUF accumulators (CCE bf16,
    parity-split mode, tokens_per_rank=128), also spread over 4 queues.
    Windows of 2048 tokens rotate over NBUF=2 accumulator sets; in-window
    duplicate rows are redirected to aux rows (256-aligned, starting at
    S_P) folded back by host-planned cleanup scatter passes.  No HBM
    read-modify-write, no accumulator zero/readback DMA traffic.
  - BN folded: scale into weights (host side), bias added in the epilogue,
    which folds the accumulator sets on DVE directly from SBUF.
  - Intermediate activation (f32) AllGather'd across cores.
"""

import sys

if "/opt/trn_rl_repo" not in sys.path:
    sys.path.insert(0, "/opt/trn_rl_repo")

import numpy as np
import ml_dtypes

BF16 = ml_dtypes.bfloat16

# ---------------------------------------------------------------- problem cfg
N = 200000  # voxels
C = 64      # channels
K = 27      # kernel offsets
M = 100000  # pairs per offset
W = 8       # cores
EPS = 1e-5

WTOK = 2048   # tokens per scatter window (one dma_scatter_add call)
TILE = 128    # tokens per matmul tile
PAIR = 256    # tokens per xbar transpose block (2 tiles)
NBUF = 2      # rotating scatter accumulator buffer sets (SBUF)
S_P = 25088   # 256-aligned padded shard rows (>= S+1); aux rows start here


# ---------------------------------------------------------------- host-side prep
def _pad128(n):
    return ((int(n) + 127) // 128) * 128


def _pad256(n):
    return ((int(n) + 255) // 256) * 256


def _weight_stacks(w_scaled):
    """[K, C, C] f32 -> [K, 2, 2C, C] bf16 with [w;0] and [0;w] stacks."""
    k, c, _ = w_scaled.shape
    wb = w_scaled.astype(BF16)
    out = np.zeros((k, 2, 2 * c, c), dtype=BF16)
    out[:, 0, :c, :] = wb
    out[:, 1, c:, :] = wb
    return np.ascontiguousarray(out)


def _prep_indices_static(in_map, out_map, n, w):
    """Deterministic two-pass version: aux rows laid out per (level, buffer)
    with uniform capacities so the device program is core-independent."""
    S = n // w
    kk = in_map.shape[0]
    TRASH = S

    owner = out_map // S
    chunk = in_map // S
    counts = np.zeros((w, kk, w), dtype=np.int64)
    for k in range(kk):
        flat = owner[k] * w + chunk[k]
        counts[:, k, :] = np.bincount(flat, minlength=w * w).reshape(w, w)
    slot_sizes = np.maximum(((counts.max(axis=0) + 255) // 256) * 256, 256)
    tot = int(slot_sizes.sum())
    n_win = (tot + WTOK - 1) // WTOK

    g_all, s_raw = [], []
    for c in range(w):
        g_stream = np.zeros(tot, dtype=np.int32)
        s_stream = np.full(tot, TRASH, dtype=np.int32)
        off = 0
        for k in range(kk):
            sel_c = owner[k] == c
            i_k = in_map[k][sel_c]
            o_k = out_map[k][sel_c] - c * S
            ch_k = chunk[k][sel_c]
            for j in range(w):
                L = int(slot_sizes[k, j])
                selj = ch_k == j
                i_loc = i_k[selj] - j * S
                o_loc = o_k[selj]
                order = np.argsort(o_loc, kind="stable")
                cnt = len(i_loc)
                g_stream[off:off + cnt] = i_loc[order]
                s_stream[off:off + cnt] = o_loc[order]
                off += L
        g_all.append(g_stream)
        s_raw.append(s_stream)

    # ---- iterative dedup with per-level uniform capacities ----
    # level 0 = main stream; dups of level l become level l+1 tokens.
    streams = [[s] for s in s_raw]             # per core: [lvl0, lvl1, ...]
    pend = [None] * w                          # per core: list[(buf, true_r)]
    lev_caps = []                              # per level: [cap_b] * NBUF
    widx0 = 0
    lvl = 0
    cur_len = tot
    while True:
        n_w = (cur_len + WTOK - 1) // WTOK
        for c in range(w):
            st = streams[c][lvl]
            pc = []
            for wi in range(n_w):
                buf = (widx0 + wi) % NBUF
                seen = set()
                a = wi * WTOK
                for t in range(a, min(a + WTOK, len(st))):
                    r = int(st[t])
                    if r >= TRASH or r < 0:
                        continue
                    if r in seen:
                        pc.append((buf, t, r))
                    else:
                        seen.add(r)
            pend[c] = pc
        widx0 += n_w
        if max(len(p) for p in pend) == 0:
            break
        cap = [0] * NBUF
        for c in range(w):
            cnt = [0] * NBUF
            for (b, t, r) in pend[c]:
                cnt[b] += 1
            for b in range(NBUF):
                cap[b] = max(cap[b], cnt[b])
        cap = [_pad256(x) if x else 0 for x in cap]
        lev_caps.append(cap)
        nlen = sum(cap)
        for c in range(w):
            st = streams[c][lvl]
            nst = np.full(nlen, TRASH, dtype=np.int32)
            loc = [0] * NBUF
            for (b, t, r) in pend[c]:
                aux_row_local = loc[b]
                loc[b] += 1
                prev = sum(lc[b] for lc in lev_caps[:-1])
                st[t] = S_P + prev + aux_row_local
                nst[sum(cap[:b]) + aux_row_local] = r
            streams[c].append(nst)
        lvl += 1
        cur_len = nlen
        assert lvl < 12

    auxcap_b = [sum(lc[b] for lc in lev_caps) for b in range(NBUF)] if lev_caps \
        else [0] * NBUF
    assert S_P + max(auxcap_b + [0]) <= 32768, auxcap_b  # max row idx 32767

    def wrap16(a):
        a = np.asarray(a, np.int16)
        assert len(a) % 16 == 0
        m16 = a.reshape(-1, 16).T.copy()
        return np.tile(m16, (8, 1))

    gidx = [wrap16(g) for g in g_all]
    sidx = [wrap16(s[0]) for s in streams]
    cidx = []
    for c in range(w):
        if lvl > 0:
            cidx.append(wrap16(np.concatenate(streams[c][1:])))
        else:
            cidx.append(np.zeros((128, 8), np.int16))

    plan = dict(slot_sizes=slot_sizes, tot=tot, lev_caps=lev_caps,
                auxcap_b=auxcap_b)
    return plan, gidx, sidx, cidx


# ---------------------------------------------------------------- device program
def build_program(n, c, k, w, plan, debug=False):
    import os
    import concourse.bacc as bacc
    import concourse.mybir as mybir
    import concourse.tile as tile

    SKIP = set(os.environ.get("KSKIP", "").split(","))

    S = n // w
    C2 = 2 * c
    dt = mybir.dt
    slot_sizes = plan["slot_sizes"]
    lev_caps = plan["lev_caps"]
    tot = plan["tot"]
    max_slot = int(slot_sizes.max())
    ctot = sum(sum(lc) for lc in lev_caps)
    n_grp_tiles = WTOK // TILE

    nc = bacc.Bacc("TRN2", target_bir_lowering=False, debug=debug, num_devices=w,
                   num_swdge_queues=4)

    # ---- I/O ----
    xs = nc.dram_tensor("xs", [n, c], dt.float32, kind="ExternalInput")
    ident = nc.dram_tensor("ident", [TILE, TILE], dt.bfloat16,
                           kind="ExternalInput")
    x_res = nc.dram_tensor("x_res", [S_P, c], dt.float32, kind="ExternalInput")
    r1 = nc.dram_tensor("r1", [k, 2, C2, c], dt.bfloat16, kind="ExternalInput")
    r2 = nc.dram_tensor("r2", [k, 2, C2, c], dt.bfloat16, kind="ExternalInput")
    b1t = nc.dram_tensor("b1t", [TILE, 8 * c], dt.float32, kind="ExternalInput")
    b2t = nc.dram_tensor("b2t", [TILE, 8 * c], dt.float32, kind="ExternalInput")
    gidx = nc.dram_tensor("gidx", [TILE, tot // 16], dt.int16, kind="ExternalInput")
    sidx = nc.dram_tensor("sidx", [TILE, tot // 16], dt.int16, kind="ExternalInput")
    cidx = nc.dram_tensor("cidx", [TILE, max(ctot, 128) // 16], dt.int16,
                          kind="ExternalInput")

    out = nc.dram_tensor("out", [S_P, c], dt.float32, kind="ExternalOutput")

    # SBUF scatter accumulators: bf16, parity-split (dma_scatter_add SBUF-dst
    # mode with tokens_per_rank=128): row r -> partition r&127, parity
    # (r>>7)&1, free-dim group r>>8.  NBUF rotating sets bound in-flight
    # windows; rows = [0,S) main + trash S + aux tail from S_P (256-aligned).
    gacc = (S_P + max(plan["auxcap_b"] + [0]) + 255) // 256

    xs1_shard = nc.dram_tensor("xs1_shard", [S_P, c], dt.float32, kind="Internal")
    xs1_full = nc.dram_tensor(
        "xs1_full", [w * S_P, c], dt.float32, kind="Internal",
        addr_space="Shared" if w > 4 else "Local",
    )

    with tile.TileContext(nc) as tc:
        with (
            tc.tile_pool(name="const", bufs=1) as const_pool,
            tc.tile_pool(name="gather", bufs=4) as gpool,
            tc.tile_pool(name="gcast", bufs=3) as gcpool,
            tc.tile_pool(name="gtra", bufs=3) as gtpool,
            tc.tile_pool(name="gi", bufs=4) as gipool,
            tc.tile_pool(name="si", bufs=4) as sipool,
            tc.tile_pool(name="stage", bufs=6) as stpool,
            tc.tile_pool(name="psum", bufs=4, space="PSUM") as ppool,
            tc.tile_pool(name="ptp", bufs=4, space="PSUM") as tppool,
            tc.tile_pool(name="ep", bufs=3) as eppool,
        ):
            # ---- constants ----
            wts = {}
            for name, t in (("r1", r1), ("r2", r2)):
                sb = const_pool.tile([C2, k * 2 * c], dt.bfloat16, tag=name)
                nc.sync.dma_start(
                    out=sb[:].rearrange("p (k h d) -> p k h d", k=k, h=2),
                    in_=t[:].rearrange("k h p d -> p k h d"),
                )
                wts[name] = sb
            b1_sb = const_pool.tile([TILE, 8 * c], dt.float32, tag="b1")
            nc.sync.dma_start(out=b1_sb[:], in_=b1t[:])
            b2_sb = const_pool.tile([TILE, 8 * c], dt.float32, tag="b2")
            nc.sync.dma_start(out=b2_sb[:], in_=b2t[:])
            id_sb = const_pool.tile([TILE, TILE], dt.bfloat16, tag="id")
            nc.sync.dma_start(out=id_sb[:], in_=ident[:])

            # ---- SBUF scatter accumulators (persistent, reused per conv) ----
            acc_sets = [
                dict(own=const_pool.tile([TILE, gacc, c], dt.bfloat16,
                                         tag=f"ac{b}o", name=f"acc{b}own"),
                     peer=const_pool.tile([TILE, gacc, c], dt.bfloat16,
                                          tag=f"ac{b}p", name=f"acc{b}peer"))
                for b in range(NBUF)
            ]

            def zero_acc():
                for s_ in acc_sets:
                    nc.vector.memset(s_["own"][:], 0.0)
                    nc.vector.memset(s_["peer"][:], 0.0)

            # ---- one sparse conv pass ----
            def conv(src_table, w_sb, cstride):
                tt = 0
                widx = 0
                stage_t = None
                psum_t = None

                def flush_group(n_tiles):
                    nonlocal widx
                    if "scat" in SKIP:
                        widx += 1
                        return
                    ntok = n_tiles * TILE
                    base16 = (tt - n_tiles) * (TILE // 16)
                    si_t = sipool.tile([TILE, WTOK // 16], dt.int16, tag="si")
                    nc.sync.dma_start(
                        out=si_t[:, : ntok // 16],
                        in_=sidx[:, base16: base16 + ntok // 16],
                    )
                    aset = acc_sets[widx % NBUF]
                    nc.gpsimd.dma_scatter_add(
                        aset["own"][:],
                        stage_t[:, : ntok * c // TILE].rearrange(
                            "p (g d) -> p g d", d=c),
                        si_t[:, : ntok // 16],
                        ntok, ntok, c,
                        sbuf_tokens_per_rank=TILE,
                        parity_reg=0,
                        out_ap_other=aset["peer"][:],
                    )
                    widx += 1

                for kk in range(k):
                    for j in range(w):
                        L = int(slot_sizes[kk, j])
                        g_t = gpool.tile([TILE, max_slot // TILE, c],
                                         dt.float32, tag="g")
                        gb_t = gcpool.tile([TILE, max_slot // TILE, c],
                                           dt.bfloat16, tag="gb")
                        gT_t = gtpool.tile([TILE, max_slot // PAIR, TILE],
                                           dt.bfloat16, tag="gT")
                        if "gath" in SKIP:
                            nc.vector.memset(g_t[:, : L // TILE, :], 0)
                        else:
                            gi_t = gipool.tile([TILE, max_slot // 16], dt.int16,
                                               tag="gi")
                            base16 = tt * (TILE // 16)
                            nc.sync.dma_start(
                                out=gi_t[:, : L // 16],
                                in_=gidx[:, base16: base16 + L // 16],
                            )
                            nc.gpsimd.dma_gather(
                                g_t[:, : L // TILE, :],
                                src_table[j * cstride: j * cstride + S, :],
                                gi_t[:, : L // 16],
                                L, L, c,
                                transpose=False,
                                single_packet=False,
                            )
                        # cast f32 -> bf16 (bulk, one ACT op per slot)
                        nc.scalar.copy(out=gb_t[:, : L // TILE, :],
                                       in_=g_t[:, : L // TILE, :])
                        # PE transpose per 256-token pair:
                        # [128 tok, 128 (2x64ch)] -> [128 (chA|chB), 128 tok]
                        for q in range(L // PAIR):
                            tp = tppool.tile([TILE, TILE], dt.bfloat16,
                                             tag="tp")
                            nc.tensor.transpose(
                                out=tp[:],
                                in_=gb_t[:, 2 * q: 2 * q + 2, :],
                                identity=id_sb[:])
                            nc.vector.tensor_copy(out=gT_t[:, q, :],
                                                  in_=tp[:])
                        for t in range(L // TILE):
                            b = tt % n_grp_tiles
                            if b == 0:
                                stage_t = stpool.tile(
                                    [TILE, WTOK * c // TILE], dt.bfloat16, tag="st")
                            if b % 8 == 0:
                                psum_t = ppool.tile([TILE, 512], dt.float32,
                                                    tag="ps")
                            ps = psum_t[:, (b % 8) * c: (b % 8 + 1) * c]
                            nc.tensor.matmul(
                                out=ps,
                                lhsT=gT_t[:, t // 2, :],
                                rhs=w_sb[:, (kk * 2 + t % 2) * c:
                                         (kk * 2 + t % 2 + 1) * c],
                                start=True, stop=True)
                            if b % 8 == 7:
                                nc.vector.tensor_copy(
                                    out=stage_t[:, (b - 7) * c: (b + 1) * c],
                                    in_=psum_t[:])
                            elif b == n_grp_tiles - 1:
                                nb = b % 8 + 1
                                nc.vector.tensor_copy(
                                    out=stage_t[:, (b + 1 - nb) * c: (b + 1) * c],
                                    in_=psum_t[:, : nb * c])
                            tt += 1
                            if tt % n_grp_tiles == 0:
                                flush_group(n_grp_tiles)
                rem = tt % n_grp_tiles
                if rem:
                    full_banks = rem // 8
                    tail = rem % 8
                    if tail:
                        nc.vector.tensor_copy(
                            out=stage_t[:, full_banks * 8 * c: rem * c],
                            in_=psum_t[:, : tail * c])
                    flush_group(rem)

                # ---- cleanup levels: fold aux rows back ----
                coff16 = 0     # offset into cidx (16ths)
                for li, cap in enumerate(lev_caps if "cleanup" not in SKIP else []):
                    lev_base = [S_P + sum(lc[bb] for lc in lev_caps[:li])
                                for bb in range(NBUF)]
                    stream_len = sum(cap)
                    n_w = (stream_len + WTOK - 1) // WTOK
                    for wi in range(n_w):
                        a = wi * WTOK
                        e = min(a + WTOK, stream_len)
                        ntok = e - a
                        st = stpool.tile([TILE, WTOK * c // TILE], dt.bfloat16,
                                         tag="st")
                        st3 = st[:].rearrange("p (g d) -> p g d", d=c)
                        for bb in range(NBUF):
                            sb0, sb1 = sum(cap[:bb]), sum(cap[:bb + 1])
                            ov0, ov1 = max(a, sb0), min(e, sb1)
                            if ov0 >= ov1:
                                continue
                            rows0 = lev_base[bb] + (ov0 - sb0)
                            n256 = (ov1 - ov0) // 256
                            g0 = rows0 // 256
                            gg0 = (ov0 - a) // TILE
                            # aux rows alternate own/peer every 128 rows,
                            # starting on an even (own) slot (256-aligned)
                            nc.vector.tensor_copy(
                                out=st3[:, gg0: gg0 + 2 * n256: 2, :],
                                in_=acc_sets[bb]["own"][:, g0: g0 + n256, :])
                            nc.vector.tensor_copy(
                                out=st3[:, gg0 + 1: gg0 + 2 * n256: 2, :],
                                in_=acc_sets[bb]["peer"][:, g0: g0 + n256, :])
                        si_t = sipool.tile([TILE, WTOK // 16], dt.int16, tag="si")
                        nc.sync.dma_start(
                            out=si_t[:, : ntok // 16],
                            in_=cidx[:, coff16 + a // 16: coff16 + e // 16],
                        )
                        aset = acc_sets[widx % NBUF]
                        nc.gpsimd.dma_scatter_add(
                            aset["own"][:],
                            st[:, : ntok * c // TILE].rearrange(
                                "p (g d) -> p g d", d=c),
                            si_t[:, : ntok // 16],
                            ntok, ntok, c,
                            sbuf_tokens_per_rank=TILE,
                            parity_reg=0,
                            out_ap_other=aset["peer"][:],
                        )
                        widx += 1
                    coff16 += stream_len // 16

            # epilogue helper: fold NBUF accumulator sets (parity-interleaved
            # 128-row blocks: even block -> own group, odd -> peer group)
            EPR = 1024
            G = EPR // TILE
            n_ep = -(-S_P // EPR)

            def fold_acc(r0):
                acc = eppool.tile([TILE, G, c], dt.float32, tag="ea")
                tmp = eppool.tile([TILE, G, c], dt.float32, tag="eb")
                g0 = r0 // 256
                for b in range(NBUF):
                    dst = acc if b == 0 else tmp
                    nc.vector.tensor_copy(
                        out=dst[:, 0::2, :],
                        in_=acc_sets[b]["own"][:, g0: g0 + G // 2, :])
                    nc.vector.tensor_copy(
                        out=dst[:, 1::2, :],
                        in_=acc_sets[b]["peer"][:, g0: g0 + G // 2, :])
                    if b > 0:
                        nc.vector.tensor_add(out=acc[:], in0=acc[:], in1=tmp[:])
                return acc

            # ======== conv1 ========
            zero_acc()
            if "conv1" not in SKIP:
                conv(xs, wts["r1"], S)

            # ======== epilogue1: fold sets + bias + relu ========
            for i in range(n_ep):
                r0 = min(i * EPR, S_P - EPR)
                acc = fold_acc(r0)
                b1v = b1_sb[:].rearrange("p (g d) -> p g d", d=c)[:, :G, :]
                nc.vector.tensor_add(out=acc[:], in0=acc[:], in1=b1v)
                nc.vector.tensor_scalar_max(acc[:], acc[:], 0.0)
                nc.sync.dma_start(
                    out=xs1_shard[r0: r0 + EPR, :].rearrange(
                        "(g p) d -> p g d", p=TILE),
                    in_=acc[:])

            # ======== allgather ========
            if w > 1 and "cc" not in SKIP:
                nc.gpsimd.collective_compute(
                    "AllGather", mybir.AluOpType.bypass,
                    replica_groups=[list(range(w))],
                    ins=[xs1_shard[:]], outs=[xs1_full[:]])
                conv2_src = xs1_full
            else:
                conv2_src = xs1_shard

            # ======== conv2 ========
            zero_acc()
            if "conv2" not in SKIP:
                conv(conv2_src, wts["r2"], S_P)

            # ======== epilogue2: fold sets + bias + residual + relu ========
            for i in range(n_ep):
                r0 = min(i * EPR, S_P - EPR)
                acc = fold_acc(r0)
                b2v = b2_sb[:].rearrange("p (g d) -> p g d", d=c)[:, :G, :]
                nc.vector.tensor_add(out=acc[:], in0=acc[:], in1=b2v)
                xr = eppool.tile([TILE, G, c], dt.float32, tag="ex")
                nc.sync.dma_start(
                    out=xr[:],
                    in_=x_res[r0: r0 + EPR, :].rearrange("(g p) d -> p g d",
                                                         p=TILE))
                nc.vector.tensor_add(out=acc[:], in0=acc[:], in1=xr[:])
                nc.vector.tensor_scalar_max(acc[:], acc[:], 0.0)
                nc.sync.dma_start(
                    out=out[r0: r0 + EPR, :].rearrange("(g p) d -> p g d",
                                                       p=TILE),
                    in_=acc[:])

    # ---- spread SWDGE desc-gen across the 4 Q7 core pairs --------------
    # Each dma_gather/dma_scatter_add is serviced by Q7 core pair
    # `queue_num` (ucode: cpu_id/2 == queue_num), so 4 queues give 4x
    # descriptor-generation throughput.  Correctness constraints: (1) the
    # xbar is a single shared resource, so only non-transpose gathers are
    # safe to spread; (2) Tile's DMASW lane semaphores count completions
    # cumulatively, which assumes FIFO completion within a lane — queue =
    # lane % 4 keeps every lane single-queue, and a queue's ring drains
    # (and fires sems) in push order, so per-lane FIFO still holds.
    from concourse.tile_sem_assignment import PROC_NAME_TO_IDX

    QMODE = os.environ.get("QMODE", "lane")
    dmasw_lane = {PROC_NAME_TO_IDX[f"DMASW{i}"]: i for i in range(8)}
    for f in nc.m.functions:
        for blk in f.blocks:
            for inst in blk.instructions:
                if not isinstance(inst, (mybir.InstDMAGatherAnt,
                                         mybir.InstDMAScatterAddAnt)):
                    continue
                lane = dmasw_lane.get(getattr(inst, "bass_scheduled_proc", None))
                if lane is None:
                    continue
                if QMODE == "lane":
                    inst.queue_num = lane % 4
                elif QMODE == "q01":
                    inst.queue_num = lane % 2
                # QMODE == "off": leave queue 0 everywhere

    nc.compile()
    return nc


# ---------------------------------------------------------------- host wrapper
def prepare(x, w1, w2, gamma1, beta1, mean1, var1, gamma2, beta2, mean2, var2,
            in_map, out_map, n=N, w=W):
    x = np.asarray(x, np.float32)
    s1 = (np.asarray(gamma1, np.float32)
          / np.sqrt(np.asarray(var1, np.float32) + EPS))
    b1 = np.asarray(beta1, np.float32) - np.asarray(mean1, np.float32) * s1
    s2 = (np.asarray(gamma2, np.float32)
          / np.sqrt(np.asarray(var2, np.float32) + EPS))
    b2 = np.asarray(beta2, np.float32) - np.asarray(mean2, np.float32) * s2

    r1 = _weight_stacks(np.asarray(w1, np.float32) * s1[None, None, :])
    r2 = _weight_stacks(np.asarray(w2, np.float32) * s2[None, None, :])

    b1_tile = np.tile(b1[None, :], (TILE, 8)).astype(np.float32)
    b2_tile = np.tile(b2[None, :], (TILE, 8)).astype(np.float32)

    plan, gidx_all, sidx_all, cidx_all = _prep_indices_static(
        np.asarray(in_map), np.asarray(out_map), n, w)

    S = n // w
    in_maps = []
    for c in range(w):
        xr_pad = np.zeros((S_P, C), np.float32)
        xr_pad[:S] = x[c * S:(c + 1) * S]
        in_maps.append(dict(
            xs=np.ascontiguousarray(x),
            ident=np.eye(TILE, dtype=BF16),
            x_res=xr_pad,
            r1=r1, r2=r2,
            b1t=b1_tile, b2t=b2_tile,
            gidx=np.ascontiguousarray(gidx_all[c]),
            sidx=np.ascontiguousarray(sidx_all[c]),
            cidx=np.ascontiguousarray(cidx_all[c]),
        ))
    return plan, in_maps


def kernel(**inputs):
    from concourse import bass_utils

    plan, in_maps = prepare(**inputs)
    nc = build_program(N, C, K, W, plan)
    res = bass_utils.run_bass_kernel_spmd(nc, in_maps, core_ids=list(range(W)))
    S = N // W
    out = np.concatenate([res.results[c]["out"][:S] for c in range(W)], axis=0)
    return out.astype(np.float32)


# revision 36
# speedup vs baseline: 1.7085x; 1.0150x over previous
"""Trainium2 Bass kernel for a MinkowskiNet BasicBlock:
    out = relu(bn2(conv(relu(bn1(conv(x, w1))), w2)) + x)
with gather-GEMM-scatter sparse convolutions over (in_map, out_map) pair lists.

Strategy (8 NeuronCores, SPMD):
  - Shard by output-voxel owner: core c owns output rows [c*S, (c+1)*S), S = N/8.
  - Replicate x (f32 [N, 64], 256B rows) and weights to all cores.
  - Gather rows with dma_gather(transpose=False): tokens land on partitions
    ([128 tok, L/128, 64] f32).  Non-transpose gathers avoid the xbar, so
    they can be spread across all 4 SWDGE queues (4 Q7 core pairs generate
    descriptors in parallel; queue = DMASW-lane % 4 keeps Tile's cumulative
    lane-semaphore accounting FIFO within each lane).
  - Per 2-tile group: ACT cast f32->bf16, then a PE transpose (matmul with
    identity) [128 tok, 128] -> [128 (chA|chB), 128 tok] feeds the GEMM
    lhsT.  (The DMA xbar is avoided entirely: it is a single shared
    resource and corrupts under multi-queue concurrency.)
  - One bf16 matmul per 128-token tile: rhs is [w;0] / [0;w] stacked per
    kernel offset, PSUM f32 accumulate, 8 tiles per PSUM bank.
  - Scatter-add with dma_scatter_add into SBUF accumulators (CCE bf16,
    parity-split mode, tokens_per_rank=128), also spread over 4 queues.
    Windows of 2048 tokens rotate over NBUF=2 accumulator sets; in-window
    duplicate rows are redirected to aux rows (256-aligned, starting at
    S_P) folded back by host-planned cleanup scatter passes.  No HBM
    read-modify-write, no accumulator zero/readback DMA traffic.
  - BN folded: scale into weights (host side), bias added in the epilogue,
    which folds the accumulator sets on DVE directly from SBUF.
  - Intermediate activation (f32) AllGather'd across cores.
"""

import sys

if "/opt/trn_rl_repo" not in sys.path:
    sys.path.insert(0, "/opt/trn_rl_repo")

import numpy as np
import ml_dtypes

BF16 = ml_dtypes.bfloat16

# ---------------------------------------------------------------- problem cfg
N = 200000  # voxels
C = 64      # channels
K = 27      # kernel offsets
M = 100000  # pairs per offset
W = 8       # cores
EPS = 1e-5

WTOK = 2048   # tokens per scatter window (one dma_scatter_add call)
TILE = 128    # tokens per matmul tile
PAIR = 256    # tokens per xbar transpose block (2 tiles)
NBUF = 2      # rotating scatter accumulator buffer sets (SBUF)
S_P = 25088   # 256-aligned padded shard rows (>= S+1); aux rows start here


# ---------------------------------------------------------------- host-side prep
def _pad128(n):
    return ((int(n) + 127) // 128) * 128


def _pad256(n):
    return ((int(n) + 255) // 256) * 256


def _weight_stacks(w_scaled):
    """[K, C, C] f32 -> [K, 2, 2C, C] bf16 with [w;0] and [0;w] stacks."""
    k, c, _ = w_scaled.shape
    wb = w_scaled.astype(BF16)
    out = np.zeros((k, 2, 2 * c, c), dtype=BF16)
    out[:, 0, :c, :] = wb
    out[:, 1, c:, :] = wb
    return np.ascontiguousarray(out)


def _prep_indices_static(in_map, out_map, n, w):
    """Deterministic two-pass version: aux rows laid out per (level, buffer)
    with uniform capacities so the device program is core-independent."""
    S = n // w
    kk = in_map.shape[0]
    TRASH = S

    owner = out_map // S
    chunk = in_map // S
    counts = np.zeros((w, kk, w), dtype=np.int64)
    for k in range(kk):
        flat = owner[k] * w + chunk[k]
        counts[:, k, :] = np.bincount(flat, minlength=w * w).reshape(w, w)
    slot_sizes = np.maximum(((counts.max(axis=0) + 255) // 256) * 256, 256)
    tot = int(slot_sizes.sum())
    n_win = (tot + WTOK - 1) // WTOK

    g_all, s_raw = [], []
    for c in range(w):
        g_stream = np.zeros(tot, dtype=np.int32)
        s_stream = np.full(tot, TRASH, dtype=np.int32)
        off = 0
        for k in range(kk):
            sel_c = owner[k] == c
            i_k = in_map[k][sel_c]
            o_k = out_map[k][sel_c] - c * S
            ch_k = chunk[k][sel_c]
            for j in range(w):
                L = int(slot_sizes[k, j])
                selj = ch_k == j
                i_loc = i_k[selj] - j * S
                o_loc = o_k[selj]
                # ascending input rows: the gather's random HBM reads become
                # a forward sweep (scatter locality is moot: SBUF dst)
                order = np.argsort(i_loc, kind="stable")
                cnt = len(i_loc)
                g_stream[off:off + cnt] = i_loc[order]
                s_stream[off:off + cnt] = o_loc[order]
                off += L
        g_all.append(g_stream)
        s_raw.append(s_stream)

    # ---- iterative dedup with per-level uniform capacities ----
    # level 0 = main stream; dups of level l become level l+1 tokens.
    streams = [[s] for s in s_raw]             # per core: [lvl0, lvl1, ...]
    pend = [None] * w                          # per core: list[(buf, true_r)]
    lev_caps = []                              # per level: [cap_b] * NBUF
    widx0 = 0
    lvl = 0
    cur_len = tot
    while True:
        n_w = (cur_len + WTOK - 1) // WTOK
        for c in range(w):
            st = streams[c][lvl]
            pc = []
            for wi in range(n_w):
                buf = (widx0 + wi) % NBUF
                seen = set()
                a = wi * WTOK
                for t in range(a, min(a + WTOK, len(st))):
                    r = int(st[t])
                    if r >= TRASH or r < 0:
                        continue
                    if r in seen:
                        pc.append((buf, t, r))
                    else:
                        seen.add(r)
            pend[c] = pc
        widx0 += n_w
        if max(len(p) for p in pend) == 0:
            break
        cap = [0] * NBUF
        for c in range(w):
            cnt = [0] * NBUF
            for (b, t, r) in pend[c]:
                cnt[b] += 1
            for b in range(NBUF):
                cap[b] = max(cap[b], cnt[b])
        cap = [_pad256(x) if x else 0 for x in cap]
        lev_caps.append(cap)
        nlen = sum(cap)
        for c in range(w):
            st = streams[c][lvl]
            nst = np.full(nlen, TRASH, dtype=np.int32)
            loc = [0] * NBUF
            for (b, t, r) in pend[c]:
                aux_row_local = loc[b]
                loc[b] += 1
                prev = sum(lc[b] for lc in lev_caps[:-1])
                st[t] = S_P + prev + aux_row_local
                nst[sum(cap[:b]) + aux_row_local] = r
            streams[c].append(nst)
        lvl += 1
        cur_len = nlen
        assert lvl < 12

    auxcap_b = [sum(lc[b] for lc in lev_caps) for b in range(NBUF)] if lev_caps \
        else [0] * NBUF
    assert S_P + max(auxcap_b + [0]) <= 32768, auxcap_b  # max row idx 32767

    def wrap16(a):
        a = np.asarray(a, np.int16)
        assert len(a) % 16 == 0
        m16 = a.reshape(-1, 16).T.copy()
        return np.tile(m16, (8, 1))

    gidx = [wrap16(g) for g in g_all]
    sidx = [wrap16(s[0]) for s in streams]
    cidx = []
    for c in range(w):
        if lvl > 0:
            cidx.append(wrap16(np.concatenate(streams[c][1:])))
        else:
            cidx.append(np.zeros((128, 8), np.int16))

    plan = dict(slot_sizes=slot_sizes, tot=tot, lev_caps=lev_caps,
                auxcap_b=auxcap_b)
    return plan, gidx, sidx, cidx


# ---------------------------------------------------------------- device program
def build_program(n, c, k, w, plan, debug=False):
    import os
    import concourse.bacc as bacc
    import concourse.mybir as mybir
    import concourse.tile as tile

    SKIP = set(os.environ.get("KSKIP", "").split(","))

    S = n // w
    C2 = 2 * c
    dt = mybir.dt
    slot_sizes = plan["slot_sizes"]
    lev_caps = plan["lev_caps"]
    tot = plan["tot"]
    max_slot = int(slot_sizes.max())
    ctot = sum(sum(lc) for lc in lev_caps)
    n_grp_tiles = WTOK // TILE

    nc = bacc.Bacc("TRN2", target_bir_lowering=False, debug=debug, num_devices=w,
                   num_swdge_queues=4)

    # ---- I/O ----
    xs = nc.dram_tensor("xs", [n, c], dt.float32, kind="ExternalInput")
    ident = nc.dram_tensor("ident", [TILE, TILE], dt.bfloat16,
                           kind="ExternalInput")
    x_res = nc.dram_tensor("x_res", [S_P, c], dt.float32, kind="ExternalInput")
    r1 = nc.dram_tensor("r1", [k, 2, C2, c], dt.bfloat16, kind="ExternalInput")
    r2 = nc.dram_tensor("r2", [k, 2, C2, c], dt.bfloat16, kind="ExternalInput")
    b1t = nc.dram_tensor("b1t", [TILE, 8 * c], dt.float32, kind="ExternalInput")
    b2t = nc.dram_tensor("b2t", [TILE, 8 * c], dt.float32, kind="ExternalInput")
    gidx = nc.dram_tensor("gidx", [TILE, tot // 16], dt.int16, kind="ExternalInput")
    sidx = nc.dram_tensor("sidx", [TILE, tot // 16], dt.int16, kind="ExternalInput")
    cidx = nc.dram_tensor("cidx", [TILE, max(ctot, 128) // 16], dt.int16,
                          kind="ExternalInput")

    out = nc.dram_tensor("out", [S_P, c], dt.float32, kind="ExternalOutput")

    # SBUF scatter accumulators: bf16, parity-split (dma_scatter_add SBUF-dst
    # mode with tokens_per_rank=128): row r -> partition r&127, parity
    # (r>>7)&1, free-dim group r>>8.  NBUF rotating sets bound in-flight
    # windows; rows = [0,S) main + trash S + aux tail from S_P (256-aligned).
    gacc = (S_P + max(plan["auxcap_b"] + [0]) + 255) // 256

    xs1_shard = nc.dram_tensor("xs1_shard", [S_P, c], dt.float32, kind="Internal")
    xs1_full = nc.dram_tensor(
        "xs1_full", [w * S_P, c], dt.float32, kind="Internal",
        addr_space="Shared" if w > 4 else "Local",
    )

    with tile.TileContext(nc) as tc:
        with (
            tc.tile_pool(name="const", bufs=1) as const_pool,
            tc.tile_pool(name="gather", bufs=4) as gpool,
            tc.tile_pool(name="gcast", bufs=3) as gcpool,
            tc.tile_pool(name="gtra", bufs=3) as gtpool,
            tc.tile_pool(name="gi", bufs=4) as gipool,
            tc.tile_pool(name="si", bufs=4) as sipool,
            tc.tile_pool(name="stage", bufs=6) as stpool,
            tc.tile_pool(name="psum", bufs=4, space="PSUM") as ppool,
            tc.tile_pool(name="ptp", bufs=4, space="PSUM") as tppool,
            tc.tile_pool(name="ep", bufs=3) as eppool,
        ):
            # ---- constants ----
            wts = {}
            for name, t in (("r1", r1), ("r2", r2)):
                sb = const_pool.tile([C2, k * 2 * c], dt.bfloat16, tag=name)
                nc.sync.dma_start(
                    out=sb[:].rearrange("p (k h d) -> p k h d", k=k, h=2),
                    in_=t[:].rearrange("k h p d -> p k h d"),
                )
                wts[name] = sb
            b1_sb = const_pool.tile([TILE, 8 * c], dt.float32, tag="b1")
            nc.sync.dma_start(out=b1_sb[:], in_=b1t[:])
            b2_sb = const_pool.tile([TILE, 8 * c], dt.float32, tag="b2")
            nc.sync.dma_start(out=b2_sb[:], in_=b2t[:])
            id_sb = const_pool.tile([TILE, TILE], dt.bfloat16, tag="id")
            nc.sync.dma_start(out=id_sb[:], in_=ident[:])

            # ---- SBUF scatter accumulators (persistent, reused per conv) ----
            acc_sets = [
                dict(own=const_pool.tile([TILE, gacc, c], dt.bfloat16,
                                         tag=f"ac{b}o", name=f"acc{b}own"),
                     peer=const_pool.tile([TILE, gacc, c], dt.bfloat16,
                                          tag=f"ac{b}p", name=f"acc{b}peer"))
                for b in range(NBUF)
            ]

            def zero_acc():
                for s_ in acc_sets:
                    nc.vector.memset(s_["own"][:], 0.0)
                    nc.vector.memset(s_["peer"][:], 0.0)

            # ---- one sparse conv pass ----
            def conv(src_table, w_sb, cstride):
                tt = 0
                widx = 0
                stage_t = None
                psum_t = None

                def flush_group(n_tiles):
                    nonlocal widx
                    if "scat" in SKIP:
                        widx += 1
                        return
                    ntok = n_tiles * TILE
                    base16 = (tt - n_tiles) * (TILE // 16)
                    si_t = sipool.tile([TILE, WTOK // 16], dt.int16, tag="si")
                    nc.sync.dma_start(
                        out=si_t[:, : ntok // 16],
                        in_=sidx[:, base16: base16 + ntok // 16],
                    )
                    aset = acc_sets[widx % NBUF]
                    nc.gpsimd.dma_scatter_add(
                        aset["own"][:],
                        stage_t[:, : ntok * c // TILE].rearrange(
                            "p (g d) -> p g d", d=c),
                        si_t[:, : ntok // 16],
                        ntok, ntok, c,
                        sbuf_tokens_per_rank=TILE,
                        parity_reg=0,
                        out_ap_other=aset["peer"][:],
                    )
                    widx += 1

                for kk in range(k):
                    for j in range(w):
                        L = int(slot_sizes[kk, j])
                        g_t = gpool.tile([TILE, max_slot // TILE, c],
                                         dt.float32, tag="g")
                        gb_t = gcpool.tile([TILE, max_slot // TILE, c],
                                           dt.bfloat16, tag="gb")
                        gT_t = gtpool.tile([TILE, max_slot // PAIR, TILE],
                                           dt.bfloat16, tag="gT")
                        if "gath" in SKIP:
                            nc.vector.memset(g_t[:, : L // TILE, :], 0)
                        else:
                            gi_t = gipool.tile([TILE, max_slot // 16], dt.int16,
                                               tag="gi")
                            base16 = tt * (TILE // 16)
                            nc.sync.dma_start(
                                out=gi_t[:, : L // 16],
                                in_=gidx[:, base16: base16 + L // 16],
                            )
                            nc.gpsimd.dma_gather(
                                g_t[:, : L // TILE, :],
                                src_table[j * cstride: j * cstride + S, :],
                                gi_t[:, : L // 16],
                                L, L, c,
                                transpose=False,
                                single_packet=False,
                            )
                        # cast f32 -> bf16 (bulk, one ACT op per slot)
                        nc.scalar.copy(out=gb_t[:, : L // TILE, :],
                                       in_=g_t[:, : L // TILE, :])
                        # PE transpose per 256-token pair:
                        # [128 tok, 128 (2x64ch)] -> [128 (chA|chB), 128 tok]
                        for q in range(L // PAIR):
                            tp = tppool.tile([TILE, TILE], dt.bfloat16,
                                             tag="tp")
                            nc.tensor.transpose(
                                out=tp[:],
                                in_=gb_t[:, 2 * q: 2 * q + 2, :],
                                identity=id_sb[:])
                            nc.vector.tensor_copy(out=gT_t[:, q, :],
                                                  in_=tp[:])
                        for t in range(L // TILE):
                            b = tt % n_grp_tiles
                            if b == 0:
                                stage_t = stpool.tile(
                                    [TILE, WTOK * c // TILE], dt.bfloat16, tag="st")
                            if b % 8 == 0:
                                psum_t = ppool.tile([TILE, 512], dt.float32,
                                                    tag="ps")
                            ps = psum_t[:, (b % 8) * c: (b % 8 + 1) * c]
                            nc.tensor.matmul(
                                out=ps,
                                lhsT=gT_t[:, t // 2, :],
                                rhs=w_sb[:, (kk * 2 + t % 2) * c:
                                         (kk * 2 + t % 2 + 1) * c],
                                start=True, stop=True)
                            if b % 8 == 7:
                                nc.vector.tensor_copy(
                                    out=stage_t[:, (b - 7) * c: (b + 1) * c],
                                    in_=psum_t[:])
                            elif b == n_grp_tiles - 1:
                                nb = b % 8 + 1
                                nc.vector.tensor_copy(
                                    out=stage_t[:, (b + 1 - nb) * c: (b + 1) * c],
                                    in_=psum_t[:, : nb * c])
                            tt += 1
                            if tt % n_grp_tiles == 0:
                                flush_group(n_grp_tiles)
                rem = tt % n_grp_tiles
                if rem:
                    full_banks = rem // 8
                    tail = rem % 8
                    if tail:
                        nc.vector.tensor_copy(
                            out=stage_t[:, full_banks * 8 * c: rem * c],
                            in_=psum_t[:, : tail * c])
                    flush_group(rem)

                # ---- cleanup levels: fold aux rows back ----
                coff16 = 0     # offset into cidx (16ths)
                for li, cap in enumerate(lev_caps if "cleanup" not in SKIP else []):
                    lev_base = [S_P + sum(lc[bb] for lc in lev_caps[:li])
                                for bb in range(NBUF)]
                    stream_len = sum(cap)
                    n_w = (stream_len + WTOK - 1) // WTOK
                    for wi in range(n_w):
                        a = wi * WTOK
                        e = min(a + WTOK, stream_len)
                        ntok = e - a
                        st = stpool.tile([TILE, WTOK * c // TILE], dt.bfloat16,
                                         tag="st")
                        st3 = st[:].rearrange("p (g d) -> p g d", d=c)
                        for bb in range(NBUF):
                            sb0, sb1 = sum(cap[:bb]), sum(cap[:bb + 1])
                            ov0, ov1 = max(a, sb0), min(e, sb1)
                            if ov0 >= ov1:
                                continue
                            rows0 = lev_base[bb] + (ov0 - sb0)
                            n256 = (ov1 - ov0) // 256
                            g0 = rows0 // 256
                            gg0 = (ov0 - a) // TILE
                            # aux rows alternate own/peer every 128 rows,
                            # starting on an even (own) slot (256-aligned)
                            nc.vector.tensor_copy(
                                out=st3[:, gg0: gg0 + 2 * n256: 2, :],
                                in_=acc_sets[bb]["own"][:, g0: g0 + n256, :])
                            nc.vector.tensor_copy(
                                out=st3[:, gg0 + 1: gg0 + 2 * n256: 2, :],
                                in_=acc_sets[bb]["peer"][:, g0: g0 + n256, :])
                        si_t = sipool.tile([TILE, WTOK // 16], dt.int16, tag="si")
                        nc.sync.dma_start(
                            out=si_t[:, : ntok // 16],
                            in_=cidx[:, coff16 + a // 16: coff16 + e // 16],
                        )
                        aset = acc_sets[widx % NBUF]
                        nc.gpsimd.dma_scatter_add(
                            aset["own"][:],
                            st[:, : ntok * c // TILE].rearrange(
                                "p (g d) -> p g d", d=c),
                            si_t[:, : ntok // 16],
                            ntok, ntok, c,
                            sbuf_tokens_per_rank=TILE,
                            parity_reg=0,
                            out_ap_other=aset["peer"][:],
                        )
                        widx += 1
                    coff16 += stream_len // 16

            # epilogue helper: fold NBUF accumulator sets (parity-interleaved
            # 128-row blocks: even block -> own group, odd -> peer group)
            EPR = 1024
            G = EPR // TILE
            n_ep = -(-S_P // EPR)

            def fold_acc(r0):
                acc = eppool.tile([TILE, G, c], dt.float32, tag="ea")
                tmp = eppool.tile([TILE, G, c], dt.float32, tag="eb")
                g0 = r0 // 256
                for b in range(NBUF):
                    dst = acc if b == 0 else tmp
                    nc.vector.tensor_copy(
                        out=dst[:, 0::2, :],
                        in_=acc_sets[b]["own"][:, g0: g0 + G // 2, :])
                    nc.vector.tensor_copy(
                        out=dst[:, 1::2, :],
                        in_=acc_sets[b]["peer"][:, g0: g0 + G // 2, :])
                    if b > 0:
                        nc.vector.tensor_add(out=acc[:], in0=acc[:], in1=tmp[:])
                return acc

            # ======== conv1 ========
            zero_acc()
            if "conv1" not in SKIP:
                conv(xs, wts["r1"], S)

            # ======== epilogue1: fold sets + bias + relu ========
            for i in range(n_ep):
                r0 = min(i * EPR, S_P - EPR)
                acc = fold_acc(r0)
                b1v = b1_sb[:].rearrange("p (g d) -> p g d", d=c)[:, :G, :]
                nc.vector.tensor_add(out=acc[:], in0=acc[:], in1=b1v)
                nc.vector.tensor_scalar_max(acc[:], acc[:], 0.0)
                nc.sync.dma_start(
                    out=xs1_shard[r0: r0 + EPR, :].rearrange(
                        "(g p) d -> p g d", p=TILE),
                    in_=acc[:])

            # ======== allgather ========
            if w > 1 and "cc" not in SKIP:
                nc.gpsimd.collective_compute(
                    "AllGather", mybir.AluOpType.bypass,
                    replica_groups=[list(range(w))],
                    ins=[xs1_shard[:]], outs=[xs1_full[:]])
                conv2_src = xs1_full
            else:
                conv2_src = xs1_shard

            # ======== conv2 ========
            zero_acc()
            if "conv2" not in SKIP:
                conv(conv2_src, wts["r2"], S_P)

            # ======== epilogue2: fold sets + bias + residual + relu ========
            for i in range(n_ep):
                r0 = min(i * EPR, S_P - EPR)
                acc = fold_acc(r0)
                b2v = b2_sb[:].rearrange("p (g d) -> p g d", d=c)[:, :G, :]
                nc.vector.tensor_add(out=acc[:], in0=acc[:], in1=b2v)
                xr = eppool.tile([TILE, G, c], dt.float32, tag="ex")
                nc.sync.dma_start(
                    out=xr[:],
                    in_=x_res[r0: r0 + EPR, :].rearrange("(g p) d -> p g d",
                                                         p=TILE))
                nc.vector.tensor_add(out=acc[:], in0=acc[:], in1=xr[:])
                nc.vector.tensor_scalar_max(acc[:], acc[:], 0.0)
                nc.sync.dma_start(
                    out=out[r0: r0 + EPR, :].rearrange("(g p) d -> p g d",
                                                       p=TILE),
                    in_=acc[:])

    # ---- spread SWDGE desc-gen across the 4 Q7 core pairs --------------
    # Each dma_gather/dma_scatter_add is serviced by Q7 core pair
    # `queue_num` (ucode: cpu_id/2 == queue_num), so 4 queues give 4x
    # descriptor-generation throughput.  Correctness constraints: (1) the
    # xbar is a single shared resource, so only non-transpose gathers are
    # safe to spread; (2) Tile's DMASW lane semaphores count completions
    # cumulatively, which assumes FIFO completion within a lane — queue =
    # lane % 4 keeps every lane single-queue, and a queue's ring drains
    # (and fires sems) in push order, so per-lane FIFO still holds.
    from concourse.tile_sem_assignment import PROC_NAME_TO_IDX

    QMODE = os.environ.get("QMODE", "lane")
    dmasw_lane = {PROC_NAME_TO_IDX[f"DMASW{i}"]: i for i in range(8)}
    for f in nc.m.functions:
        for blk in f.blocks:
            for inst in blk.instructions:
                if not isinstance(inst, (mybir.InstDMAGatherAnt,
                                         mybir.InstDMAScatterAddAnt)):
                    continue
                lane = dmasw_lane.get(getattr(inst, "bass_scheduled_proc", None))
                if lane is None:
                    continue
                if QMODE == "lane":
                    inst.queue_num = lane % 4
                elif QMODE == "q01":
                    inst.queue_num = lane % 2
                # QMODE == "off": leave queue 0 everywhere

    nc.compile()
    return nc


# ---------------------------------------------------------------- host wrapper
def prepare(x, w1, w2, gamma1, beta1, mean1, var1, gamma2, beta2, mean2, var2,
            in_map, out_map, n=N, w=W):
    x = np.asarray(x, np.float32)
    s1 = (np.asarray(gamma1, np.float32)
          / np.sqrt(np.asarray(var1, np.float32) + EPS))
    b1 = np.asarray(beta1, np.float32) - np.asarray(mean1, np.float32) * s1
    s2 = (np.asarray(gamma2, np.float32)
          / np.sqrt(np.asarray(var2, np.float32) + EPS))
    b2 = np.asarray(beta2, np.float32) - np.asarray(mean2, np.float32) * s2

    r1 = _weight_stacks(np.asarray(w1, np.float32) * s1[None, None, :])
    r2 = _weight_stacks(np.asarray(w2, np.float32) * s2[None, None, :])

    b1_tile = np.tile(b1[None, :], (TILE, 8)).astype(np.float32)
    b2_tile = np.tile(b2[None, :], (TILE, 8)).astype(np.float32)

    plan, gidx_all, sidx_all, cidx_all = _prep_indices_static(
        np.asarray(in_map), np.asarray(out_map), n, w)

    S = n // w
    in_maps = []
    for c in range(w):
        xr_pad = np.zeros((S_P, C), np.float32)
        xr_pad[:S] = x[c * S:(c + 1) * S]
        in_maps.append(dict(
            xs=np.ascontiguousarray(x),
            ident=np.eye(TILE, dtype=BF16),
            x_res=xr_pad,
            r1=r1, r2=r2,
            b1t=b1_tile, b2t=b2_tile,
            gidx=np.ascontiguousarray(gidx_all[c]),
            sidx=np.ascontiguousarray(sidx_all[c]),
            cidx=np.ascontiguousarray(cidx_all[c]),
        ))
    return plan, in_maps


def kernel(**inputs):
    from concourse import bass_utils

    plan, in_maps = prepare(**inputs)
    nc = build_program(N, C, K, W, plan)
    res = bass_utils.run_bass_kernel_spmd(nc, in_maps, core_ids=list(range(W)))
    S = N // W
    out = np.concatenate([res.results[c]["out"][:S] for c in range(W)], axis=0)
    return out.astype(np.float32)


# revision 38
# speedup vs baseline: 1.8012x; 1.0543x over previous
"""Trainium2 Bass kernel for a MinkowskiNet BasicBlock:
    out = relu(bn2(conv(relu(bn1(conv(x, w1))), w2)) + x)
with gather-GEMM-scatter sparse convolutions over (in_map, out_map) pair lists.

Strategy (8 NeuronCores, SPMD):
  - Shard by output-voxel owner: core c owns output rows [c*S, (c+1)*S), S = N/8.
  - Replicate x (f32 [N, 64], 256B rows) and weights to all cores.
  - Gather rows with dma_gather(transpose=False): tokens land on partitions
    ([128 tok, L/128, 64] f32).  Non-transpose gathers avoid the xbar, so
    they can be spread across all 4 SWDGE queues (4 Q7 core pairs generate
    descriptors in parallel; queue = DMASW-lane % 4 keeps Tile's cumulative
    lane-semaphore accounting FIFO within each lane).
  - Per 2-tile group: ACT cast f32->bf16, then a PE transpose (matmul with
    identity) [128 tok, 128] -> [128 (chA|chB), 128 tok] feeds the GEMM
    lhsT.  (The DMA xbar is avoided entirely: it is a single shared
    resource and corrupts under multi-queue concurrency.)
  - One bf16 matmul per 128-token tile: rhs is [w;0] / [0;w] stacked per
    kernel offset, PSUM f32 accumulate, 8 tiles per PSUM bank.
  - Scatter-add with dma_scatter_add into SBUF accumulators (CCE bf16,
    parity-split mode, tokens_per_rank=128), also spread over 4 queues.
    Windows of 2048 tokens rotate over NBUF=2 accumulator sets; in-window
    duplicate rows are redirected to aux rows (256-aligned, starting at
    S_P) folded back by host-planned cleanup scatter passes.  No HBM
    read-modify-write, no accumulator zero/readback DMA traffic.
  - BN folded: scale into weights (host side), bias added in the epilogue,
    which folds the accumulator sets on DVE directly from SBUF.
  - Intermediate activation (f32) AllGather'd across cores.
"""

import sys

if "/opt/trn_rl_repo" not in sys.path:
    sys.path.insert(0, "/opt/trn_rl_repo")

import numpy as np
import ml_dtypes

BF16 = ml_dtypes.bfloat16

# ---------------------------------------------------------------- problem cfg
N = 200000  # voxels
C = 64      # channels
K = 27      # kernel offsets
M = 100000  # pairs per offset
W = 8       # cores
EPS = 1e-5

WTOK = 2048   # tokens per scatter window (one dma_scatter_add call)
TILE = 128    # tokens per matmul tile
PAIR = 256    # tokens per xbar transpose block (2 tiles)
NBUF = 2      # rotating scatter accumulator buffer sets (SBUF)
S_P = 25088   # 256-aligned padded shard rows (>= S+1); aux rows start here


# ---------------------------------------------------------------- host-side prep
def _pad128(n):
    return ((int(n) + 127) // 128) * 128


def _pad256(n):
    return ((int(n) + 255) // 256) * 256


def _weight_stacks(w_scaled):
    """[K, C, C] f32 -> [K, 2, 2C, C] bf16 with [w;0] and [0;w] stacks."""
    k, c, _ = w_scaled.shape
    wb = w_scaled.astype(BF16)
    out = np.zeros((k, 2, 2 * c, c), dtype=BF16)
    out[:, 0, :c, :] = wb
    out[:, 1, c:, :] = wb
    return np.ascontiguousarray(out)


def _prep_indices_static(in_map, out_map, n, w):
    """Deterministic two-pass version: aux rows laid out per (level, buffer)
    with uniform capacities so the device program is core-independent."""
    S = n // w
    kk = in_map.shape[0]
    TRASH = S

    owner = out_map // S
    chunk = in_map // S
    counts = np.zeros((w, kk, w), dtype=np.int64)
    for k in range(kk):
        flat = owner[k] * w + chunk[k]
        counts[:, k, :] = np.bincount(flat, minlength=w * w).reshape(w, w)
    slot_sizes = np.maximum(((counts.max(axis=0) + 255) // 256) * 256, 256)
    tot = int(slot_sizes.sum())
    n_win = (tot + WTOK - 1) // WTOK

    g_all, s_raw = [], []
    for c in range(w):
        g_stream = np.zeros(tot, dtype=np.int32)
        s_stream = np.full(tot, TRASH, dtype=np.int32)
        off = 0
        for k in range(kk):
            sel_c = owner[k] == c
            i_k = in_map[k][sel_c]
            o_k = out_map[k][sel_c] - c * S
            ch_k = chunk[k][sel_c]
            for j in range(w):
                L = int(slot_sizes[k, j])
                selj = ch_k == j
                i_loc = i_k[selj] - j * S
                o_loc = o_k[selj]
                # ascending input rows: the gather's random HBM reads become
                # a forward sweep (scatter locality is moot: SBUF dst)
                order = np.argsort(i_loc, kind="stable")
                cnt = len(i_loc)
                g_stream[off:off + cnt] = i_loc[order]
                s_stream[off:off + cnt] = o_loc[order]
                off += L
        g_all.append(g_stream)
        s_raw.append(s_stream)

    # ---- iterative dedup with per-level uniform capacities ----
    # level 0 = main stream; dups of level l become level l+1 tokens.
    streams = [[s] for s in s_raw]             # per core: [lvl0, lvl1, ...]
    pend = [None] * w                          # per core: list[(buf, true_r)]
    lev_caps = []                              # per level: [cap_b] * NBUF
    widx0 = 0
    lvl = 0
    cur_len = tot
    while True:
        n_w = (cur_len + WTOK - 1) // WTOK
        for c in range(w):
            st = streams[c][lvl]
            pc = []
            for wi in range(n_w):
                buf = (widx0 + wi) % NBUF
                seen = set()
                a = wi * WTOK
                for t in range(a, min(a + WTOK, len(st))):
                    r = int(st[t])
                    if r >= TRASH or r < 0:
                        continue
                    if r in seen:
                        pc.append((buf, t, r))
                    else:
                        seen.add(r)
            pend[c] = pc
        widx0 += n_w
        if max(len(p) for p in pend) == 0:
            break
        cap = [0] * NBUF
        for c in range(w):
            cnt = [0] * NBUF
            for (b, t, r) in pend[c]:
                cnt[b] += 1
            for b in range(NBUF):
                cap[b] = max(cap[b], cnt[b])
        cap = [_pad256(x) if x else 0 for x in cap]
        lev_caps.append(cap)
        nlen = sum(cap)
        for c in range(w):
            st = streams[c][lvl]
            nst = np.full(nlen, TRASH, dtype=np.int32)
            loc = [0] * NBUF
            for (b, t, r) in pend[c]:
                aux_row_local = loc[b]
                loc[b] += 1
                prev = sum(lc[b] for lc in lev_caps[:-1])
                st[t] = S_P + prev + aux_row_local
                nst[sum(cap[:b]) + aux_row_local] = r
            streams[c].append(nst)
        lvl += 1
        cur_len = nlen
        assert lvl < 12

    auxcap_b = [sum(lc[b] for lc in lev_caps) for b in range(NBUF)] if lev_caps \
        else [0] * NBUF
    assert S_P + max(auxcap_b + [0]) <= 32768, auxcap_b  # max row idx 32767

    def wrap16(a):
        a = np.asarray(a, np.int16)
        assert len(a) % 16 == 0
        m16 = a.reshape(-1, 16).T.copy()
        return np.tile(m16, (8, 1))

    gidx = [wrap16(g) for g in g_all]
    sidx = [wrap16(s[0]) for s in streams]
    cidx = []
    for c in range(w):
        if lvl > 0:
            cidx.append(wrap16(np.concatenate(streams[c][1:])))
        else:
            cidx.append(np.zeros((128, 8), np.int16))

    plan = dict(slot_sizes=slot_sizes, tot=tot, lev_caps=lev_caps,
                auxcap_b=auxcap_b)
    return plan, gidx, sidx, cidx


# ---------------------------------------------------------------- device program
def build_program(n, c, k, w, plan, debug=False):
    import os
    import concourse.bacc as bacc
    import concourse.mybir as mybir
    import concourse.tile as tile

    SKIP = set(os.environ.get("KSKIP", "").split(","))

    S = n // w
    C2 = 2 * c
    dt = mybir.dt
    slot_sizes = plan["slot_sizes"]
    lev_caps = plan["lev_caps"]
    tot = plan["tot"]
    max_slot = int(slot_sizes.max())
    ctot = sum(sum(lc) for lc in lev_caps)
    n_grp_tiles = WTOK // TILE

    nc = bacc.Bacc("TRN2", target_bir_lowering=False, debug=debug, num_devices=w,
                   num_swdge_queues=4, dynamic_dma_scratch_size=32768)

    # ---- I/O ----
    xs = nc.dram_tensor("xs", [n, c], dt.float32, kind="ExternalInput")
    ident = nc.dram_tensor("ident", [TILE, TILE], dt.bfloat16,
                           kind="ExternalInput")
    x_res = nc.dram_tensor("x_res", [S_P, c], dt.float32, kind="ExternalInput")
    r1 = nc.dram_tensor("r1", [k, 2, C2, c], dt.bfloat16, kind="ExternalInput")
    r2 = nc.dram_tensor("r2", [k, 2, C2, c], dt.bfloat16, kind="ExternalInput")
    b1t = nc.dram_tensor("b1t", [TILE, 8 * c], dt.float32, kind="ExternalInput")
    b2t = nc.dram_tensor("b2t", [TILE, 8 * c], dt.float32, kind="ExternalInput")
    gidx = nc.dram_tensor("gidx", [TILE, tot // 16], dt.int16, kind="ExternalInput")
    sidx = nc.dram_tensor("sidx", [TILE, tot // 16], dt.int16, kind="ExternalInput")
    cidx = nc.dram_tensor("cidx", [TILE, max(ctot, 128) // 16], dt.int16,
                          kind="ExternalInput")

    out = nc.dram_tensor("out", [S_P, c], dt.float32, kind="ExternalOutput")

    # SBUF scatter accumulators: bf16, parity-split (dma_scatter_add SBUF-dst
    # mode with tokens_per_rank=128): row r -> partition r&127, parity
    # (r>>7)&1, free-dim group r>>8.  NBUF rotating sets bound in-flight
    # windows; rows = [0,S) main + trash S + aux tail from S_P (256-aligned).
    gacc = (S_P + max(plan["auxcap_b"] + [0]) + 255) // 256

    xs1_shard = nc.dram_tensor("xs1_shard", [S_P, c], dt.float32, kind="Internal")
    xs1_full = nc.dram_tensor(
        "xs1_full", [w * S_P, c], dt.float32, kind="Internal",
        addr_space="Shared" if w > 4 else "Local",
    )

    with tile.TileContext(nc) as tc:
        with (
            tc.tile_pool(name="const", bufs=1) as const_pool,
            tc.tile_pool(name="gather", bufs=5) as gpool,
            tc.tile_pool(name="gcast", bufs=3) as gcpool,
            tc.tile_pool(name="gtra", bufs=3) as gtpool,
            tc.tile_pool(name="gi", bufs=6) as gipool,
            tc.tile_pool(name="si", bufs=4) as sipool,
            tc.tile_pool(name="stage", bufs=6) as stpool,
            tc.tile_pool(name="psum", bufs=4, space="PSUM") as ppool,
            tc.tile_pool(name="ptp", bufs=4, space="PSUM") as tppool,
            tc.tile_pool(name="ep", bufs=3) as eppool,
        ):
            # ---- constants ----
            wts = {}
            for name, t in (("r1", r1), ("r2", r2)):
                sb = const_pool.tile([C2, k * 2 * c], dt.bfloat16, tag=name)
                nc.sync.dma_start(
                    out=sb[:].rearrange("p (k h d) -> p k h d", k=k, h=2),
                    in_=t[:].rearrange("k h p d -> p k h d"),
                )
                wts[name] = sb
            b1_sb = const_pool.tile([TILE, 8 * c], dt.float32, tag="b1")
            nc.sync.dma_start(out=b1_sb[:], in_=b1t[:])
            b2_sb = const_pool.tile([TILE, 8 * c], dt.float32, tag="b2")
            nc.sync.dma_start(out=b2_sb[:], in_=b2t[:])
            id_sb = const_pool.tile([TILE, TILE], dt.bfloat16, tag="id")
            nc.sync.dma_start(out=id_sb[:], in_=ident[:])

            # ---- SBUF scatter accumulators (persistent, reused per conv) ----
            acc_sets = [
                dict(own=const_pool.tile([TILE, gacc, c], dt.bfloat16,
                                         tag=f"ac{b}o", name=f"acc{b}own"),
                     peer=const_pool.tile([TILE, gacc, c], dt.bfloat16,
                                          tag=f"ac{b}p", name=f"acc{b}peer"))
                for b in range(NBUF)
            ]

            def zero_acc():
                for s_ in acc_sets:
                    nc.vector.memset(s_["own"][:], 0.0)
                    nc.vector.memset(s_["peer"][:], 0.0)

            # ---- one sparse conv pass ----
            def conv(src_table, w_sb, cstride):
                tt = 0
                widx = 0
                stage_t = None
                psum_t = None

                def flush_group(n_tiles):
                    nonlocal widx
                    if "scat" in SKIP:
                        widx += 1
                        return
                    ntok = n_tiles * TILE
                    base16 = (tt - n_tiles) * (TILE // 16)
                    si_t = sipool.tile([TILE, WTOK // 16], dt.int16, tag="si")
                    nc.sync.dma_start(
                        out=si_t[:, : ntok // 16],
                        in_=sidx[:, base16: base16 + ntok // 16],
                    )
                    aset = acc_sets[widx % NBUF]
                    nc.gpsimd.dma_scatter_add(
                        aset["own"][:],
                        stage_t[:, : ntok * c // TILE].rearrange(
                            "p (g d) -> p g d", d=c),
                        si_t[:, : ntok // 16],
                        ntok, ntok, c,
                        sbuf_tokens_per_rank=TILE,
                        parity_reg=0,
                        out_ap_other=aset["peer"][:],
                    )
                    widx += 1

                for kk in range(k):
                    for j in range(w):
                        L = int(slot_sizes[kk, j])
                        g_t = gpool.tile([TILE, max_slot // TILE, c],
                                         dt.float32, tag="g")
                        gb_t = gcpool.tile([TILE, max_slot // TILE, c],
                                           dt.bfloat16, tag="gb")
                        gT_t = gtpool.tile([TILE, max_slot // PAIR, TILE],
                                           dt.bfloat16, tag="gT")
                        if "gath" in SKIP:
                            nc.vector.memset(g_t[:, : L // TILE, :], 0)
                        else:
                            gi_t = gipool.tile([TILE, max_slot // 16], dt.int16,
                                               tag="gi")
                            base16 = tt * (TILE // 16)
                            nc.sync.dma_start(
                                out=gi_t[:, : L // 16],
                                in_=gidx[:, base16: base16 + L // 16],
                            )
                            nc.gpsimd.dma_gather(
                                g_t[:, : L // TILE, :],
                                src_table[j * cstride: j * cstride + S, :],
                                gi_t[:, : L // 16],
                                L, L, c,
                                transpose=False,
                                single_packet=False,
                            )
                        # cast f32 -> bf16 (bulk, one ACT op per slot)
                        nc.scalar.copy(out=gb_t[:, : L // TILE, :],
                                       in_=g_t[:, : L // TILE, :])
                        # PE transpose per 256-token pair:
                        # [128 tok, 128 (2x64ch)] -> [128 (chA|chB), 128 tok]
                        for q in range(L // PAIR):
                            tp = tppool.tile([TILE, TILE], dt.bfloat16,
                                             tag="tp")
                            nc.tensor.transpose(
                                out=tp[:],
                                in_=gb_t[:, 2 * q: 2 * q + 2, :],
                                identity=id_sb[:])
                            nc.vector.tensor_copy(out=gT_t[:, q, :],
                                                  in_=tp[:])
                        for t in range(L // TILE):
                            b = tt % n_grp_tiles
                            if b == 0:
                                stage_t = stpool.tile(
                                    [TILE, WTOK * c // TILE], dt.bfloat16, tag="st")
                            if b % 8 == 0:
                                psum_t = ppool.tile([TILE, 512], dt.float32,
                                                    tag="ps")
                            ps = psum_t[:, (b % 8) * c: (b % 8 + 1) * c]
                            nc.tensor.matmul(
                                out=ps,
                                lhsT=gT_t[:, t // 2, :],
                                rhs=w_sb[:, (kk * 2 + t % 2) * c:
                                         (kk * 2 + t % 2 + 1) * c],
                                start=True, stop=True)
                            if b % 8 == 7:
                                nc.vector.tensor_copy(
                                    out=stage_t[:, (b - 7) * c: (b + 1) * c],
                                    in_=psum_t[:])
                            elif b == n_grp_tiles - 1:
                                nb = b % 8 + 1
                                nc.vector.tensor_copy(
                                    out=stage_t[:, (b + 1 - nb) * c: (b + 1) * c],
                                    in_=psum_t[:, : nb * c])
                            tt += 1
                            if tt % n_grp_tiles == 0:
                                flush_group(n_grp_tiles)
                rem = tt % n_grp_tiles
                if rem:
                    full_banks = rem // 8
                    tail = rem % 8
                    if tail:
                        nc.vector.tensor_copy(
                            out=stage_t[:, full_banks * 8 * c: rem * c],
                            in_=psum_t[:, : tail * c])
                    flush_group(rem)

                # ---- cleanup levels: fold aux rows back ----
                coff16 = 0     # offset into cidx (16ths)
                for li, cap in enumerate(lev_caps if "cleanup" not in SKIP else []):
                    lev_base = [S_P + sum(lc[bb] for lc in lev_caps[:li])
                                for bb in range(NBUF)]
                    stream_len = sum(cap)
                    n_w = (stream_len + WTOK - 1) // WTOK
                    for wi in range(n_w):
                        a = wi * WTOK
                        e = min(a + WTOK, stream_len)
                        ntok = e - a
                        st = stpool.tile([TILE, WTOK * c // TILE], dt.bfloat16,
                                         tag="st")
                        st3 = st[:].rearrange("p (g d) -> p g d", d=c)
                        for bb in range(NBUF):
                            sb0, sb1 = sum(cap[:bb]), sum(cap[:bb + 1])
                            ov0, ov1 = max(a, sb0), min(e, sb1)
                            if ov0 >= ov1:
                                continue
                            rows0 = lev_base[bb] + (ov0 - sb0)
                            n256 = (ov1 - ov0) // 256
                            g0 = rows0 // 256
                            gg0 = (ov0 - a) // TILE
                            # aux rows alternate own/peer every 128 rows,
                            # starting on an even (own) slot (256-aligned)
                            nc.vector.tensor_copy(
                                out=st3[:, gg0: gg0 + 2 * n256: 2, :],
                                in_=acc_sets[bb]["own"][:, g0: g0 + n256, :])
                            nc.vector.tensor_copy(
                                out=st3[:, gg0 + 1: gg0 + 2 * n256: 2, :],
                                in_=acc_sets[bb]["peer"][:, g0: g0 + n256, :])
                        si_t = sipool.tile([TILE, WTOK // 16], dt.int16, tag="si")
                        nc.sync.dma_start(
                            out=si_t[:, : ntok // 16],
                            in_=cidx[:, coff16 + a // 16: coff16 + e // 16],
                        )
                        aset = acc_sets[widx % NBUF]
                        nc.gpsimd.dma_scatter_add(
                            aset["own"][:],
                            st[:, : ntok * c // TILE].rearrange(
                                "p (g d) -> p g d", d=c),
                            si_t[:, : ntok // 16],
                            ntok, ntok, c,
                            sbuf_tokens_per_rank=TILE,
                            parity_reg=0,
                            out_ap_other=aset["peer"][:],
                        )
                        widx += 1
                    coff16 += stream_len // 16

            # epilogue helper: fold NBUF accumulator sets (parity-interleaved
            # 128-row blocks: even block -> own group, odd -> peer group)
            EPR = 1024
            G = EPR // TILE
            n_ep = -(-S_P // EPR)

            def fold_acc(r0):
                acc = eppool.tile([TILE, G, c], dt.float32, tag="ea")
                tmp = eppool.tile([TILE, G, c], dt.float32, tag="eb")
                g0 = r0 // 256
                for b in range(NBUF):
                    dst = acc if b == 0 else tmp
                    nc.vector.tensor_copy(
                        out=dst[:, 0::2, :],
                        in_=acc_sets[b]["own"][:, g0: g0 + G // 2, :])
                    nc.vector.tensor_copy(
                        out=dst[:, 1::2, :],
                        in_=acc_sets[b]["peer"][:, g0: g0 + G // 2, :])
                    if b > 0:
                        nc.vector.tensor_add(out=acc[:], in0=acc[:], in1=tmp[:])
                return acc

            # ======== conv1 ========
            zero_acc()
            if "conv1" not in SKIP:
                conv(xs, wts["r1"], S)

            # ======== epilogue1: fold sets + bias + relu ========
            for i in range(n_ep):
                r0 = min(i * EPR, S_P - EPR)
                acc = fold_acc(r0)
                b1v = b1_sb[:].rearrange("p (g d) -> p g d", d=c)[:, :G, :]
                nc.vector.tensor_add(out=acc[:], in0=acc[:], in1=b1v)
                nc.vector.tensor_scalar_max(acc[:], acc[:], 0.0)
                nc.sync.dma_start(
                    out=xs1_shard[r0: r0 + EPR, :].rearrange(
                        "(g p) d -> p g d", p=TILE),
                    in_=acc[:])

            # ======== allgather ========
            if w > 1 and "cc" not in SKIP:
                nc.gpsimd.collective_compute(
                    "AllGather", mybir.AluOpType.bypass,
                    replica_groups=[list(range(w))],
                    ins=[xs1_shard[:]], outs=[xs1_full[:]])
                conv2_src = xs1_full
            else:
                conv2_src = xs1_shard

            # ======== conv2 ========
            zero_acc()
            if "conv2" not in SKIP:
                conv(conv2_src, wts["r2"], S_P)

            # ======== epilogue2: fold sets + bias + residual + relu ========
            for i in range(n_ep):
                r0 = min(i * EPR, S_P - EPR)
                acc = fold_acc(r0)
                b2v = b2_sb[:].rearrange("p (g d) -> p g d", d=c)[:, :G, :]
                nc.vector.tensor_add(out=acc[:], in0=acc[:], in1=b2v)
                xr = eppool.tile([TILE, G, c], dt.float32, tag="ex")
                nc.sync.dma_start(
                    out=xr[:],
                    in_=x_res[r0: r0 + EPR, :].rearrange("(g p) d -> p g d",
                                                         p=TILE))
                nc.vector.tensor_add(out=acc[:], in0=acc[:], in1=xr[:])
                nc.vector.tensor_scalar_max(acc[:], acc[:], 0.0)
                nc.sync.dma_start(
                    out=out[r0: r0 + EPR, :].rearrange("(g p) d -> p g d",
                                                       p=TILE),
                    in_=acc[:])

    # ---- spread SWDGE desc-gen across the 4 Q7 core pairs --------------
    # Each dma_gather/dma_scatter_add is serviced by Q7 core pair
    # `queue_num` (ucode: cpu_id/2 == queue_num), so 4 queues give 4x
    # descriptor-generation throughput.  Correctness constraints: (1) the
    # xbar is a single shared resource, so only non-transpose gathers are
    # safe to spread; (2) Tile's DMASW lane semaphores count completions
    # cumulatively, which assumes FIFO completion within a lane — queue =
    # lane % 4 keeps every lane single-queue, and a queue's ring drains
    # (and fires sems) in push order, so per-lane FIFO still holds.
    from concourse.tile_sem_assignment import PROC_NAME_TO_IDX

    QMODE = os.environ.get("QMODE", "lane")
    dmasw_lane = {PROC_NAME_TO_IDX[f"DMASW{i}"]: i for i in range(8)}
    for f in nc.m.functions:
        for blk in f.blocks:
            for inst in blk.instructions:
                if not isinstance(inst, (mybir.InstDMAGatherAnt,
                                         mybir.InstDMAScatterAddAnt)):
                    continue
                lane = dmasw_lane.get(getattr(inst, "bass_scheduled_proc", None))
                if lane is None:
                    continue
                if QMODE == "lane":
                    inst.queue_num = lane % 4
                elif QMODE == "q01":
                    inst.queue_num = lane % 2
                # QMODE == "off": leave queue 0 everywhere

    nc.compile()
    return nc


# ---------------------------------------------------------------- host wrapper
def prepare(x, w1, w2, gamma1, beta1, mean1, var1, gamma2, beta2, mean2, var2,
            in_map, out_map, n=N, w=W):
    x = np.asarray(x, np.float32)
    s1 = (np.asarray(gamma1, np.float32)
          / np.sqrt(np.asarray(var1, np.float32) + EPS))
    b1 = np.asarray(beta1, np.float32) - np.asarray(mean1, np.float32) * s1
    s2 = (np.asarray(gamma2, np.float32)
          / np.sqrt(np.asarray(var2, np.float32) + EPS))
    b2 = np.asarray(beta2, np.float32) - np.asarray(mean2, np.float32) * s2

    r1 = _weight_stacks(np.asarray(w1, np.float32) * s1[None, None, :])
    r2 = _weight_stacks(np.asarray(w2, np.float32) * s2[None, None, :])

    b1_tile = np.tile(b1[None, :], (TILE, 8)).astype(np.float32)
    b2_tile = np.tile(b2[None, :], (TILE, 8)).astype(np.float32)

    plan, gidx_all, sidx_all, cidx_all = _prep_indices_static(
        np.asarray(in_map), np.asarray(out_map), n, w)

    S = n // w
    in_maps = []
    for c in range(w):
        xr_pad = np.zeros((S_P, C), np.float32)
        xr_pad[:S] = x[c * S:(c + 1) * S]
        in_maps.append(dict(
            xs=np.ascontiguousarray(x),
            ident=np.eye(TILE, dtype=BF16),
            x_res=xr_pad,
            r1=r1, r2=r2,
            b1t=b1_tile, b2t=b2_tile,
            gidx=np.ascontiguousarray(gidx_all[c]),
            sidx=np.ascontiguousarray(sidx_all[c]),
            cidx=np.ascontiguousarray(cidx_all[c]),
        ))
    return plan, in_maps


def kernel(**inputs):
    from concourse import bass_utils

    plan, in_maps = prepare(**inputs)
    nc = build_program(N, C, K, W, plan)
    res = bass_utils.run_bass_kernel_spmd(nc, in_maps, core_ids=list(range(W)))
    S = N // W
    out = np.concatenate([res.results[c]["out"][:S] for c in range(W)], axis=0)
    return out.astype(np.float32)
